# revision 11
# baseline (speedup 1.0000x reference)
"""Expert-parallel MoE (Kimi/DeepSeek-V3 sparse block) on 8 trn2 NeuronCores.

Strategy (v2):
  - Host computes the sigmoid gate + group-limited top-2 routing in float64
    and gathers each expert's tokens into a transposed, k-interleaved batch.
  - Capacity R = align32(max expert load) is chosen from the *actual*
    routing, so per-core work is R + 2*512 shared token-instances instead of
    a fixed 1536 + 1024 — the program is compiled per capacity and cached.
  - All matmul operands are bf16 (fp32 PSUM accumulate): halves HBM->SBUF
    traffic and enables fast weight load; error stays ~1e-3 vs the 2e-2 gate.
  - Weight/x DMAs are emitted as per-k-slice transfers interleaved so the
    first matmul can start after ~400KB instead of ~8MB; the first chunk's
    up-phase runs k-outer to consume slices in arrival order.
  - Up-phase is mf-outer afterwards (silu/mul overlap the next h-pair);
    down-phase is kf-outer so the PE never waits on the last ht tile.
  - Outputs stream back on the vector engine's DMA queue to keep the sync
    queue free for input prefetch.
"""

from contextlib import ExitStack

import numpy as np
import ml_dtypes

import concourse.bacc as bacc
import concourse.tile as tile
import concourse.mybir as mybir
from concourse import bass_utils

# --- model dims (hardcoded per problem spec) ---
B, S, D = 2, 2048, 1024
T = B * S                 # 4096 tokens
E, F = 8, 512             # routed experts / expert intermediate
SH = 1024                 # shared intermediate (= 2 halves of F)
TOP_K, N_GROUP, TOPK_GROUP = 2, 4, 2
SCALE = 2.5

N_CORES = 8
P = 128                   # SBUF partitions
KD = D // P               # 8 contraction tiles over D
KF = F // P               # 4 contraction tiles over F
NT = 512                  # max matmul moving free dim (one PSUM bank fp32)
SHT = T // N_CORES        # 512 shared-expert tokens per core
R_MAX = 4096              # capacity ceiling (SBUF); overflow -> host

F32 = mybir.dt.float32
BF16 = mybir.dt.bfloat16
BF16_NP = ml_dtypes.bfloat16

_CACHE: dict = {}


def _chunk_plan(R):
    """Chunks: (mode, wset, col_off, ncols).

    Order: first expert chunk (k-outer DMA streaming), then the two shared
    passes, then remaining expert chunks — the small expert remainder lands
    last so the post-matmul tail (copies + output DMA) is short."""
    echunks = []
    c = 0
    while c < R:
        n = min(NT, R - c)
        echunks.append(("e", 0, c, n))
        c += n
    schunks = []
    for mode, s in (("s0", 1), ("s1", 2)):
        c = 0
        while c < SHT:
            n = min(NT, SHT - c)
            schunks.append((mode, s, c, n))
            c += n
    return tuple(echunks[:1] + schunks + echunks[1:])


def _emit(nc, R):
    TOT = R + SHT
    chunks = _chunk_plan(R)
    n0 = chunks[0][3]  # first chunk size

    xall = nc.dram_tensor("xall", [P, KD, TOT], BF16, kind="ExternalInput").ap()
    wd = []
    for s in range(3):
        w1 = nc.dram_tensor(f"w1_{s}", [P, KD, F], BF16, kind="ExternalInput").ap()
        w3 = nc.dram_tensor(f"w3_{s}", [P, KD, F], BF16, kind="ExternalInput").ap()
        w2 = nc.dram_tensor(f"w2_{s}", [P, KF, D], BF16, kind="ExternalInput").ap()
        wd.append((w1, w3, w2))
    yall = nc.dram_tensor("yall", [D, TOT], F32, kind="ExternalOutput").ap()
    yallr = yall.rearrange("(k p) n -> p k n", p=P)  # (p,k,n) AP for SBUF-order DMA

    silu = mybir.ActivationFunctionType.Silu

    with tile.TileContext(nc) as tc, ExitStack() as ctx:
        wpool = ctx.enter_context(tc.tile_pool(name="wpool", bufs=1))
        xpool = ctx.enter_context(tc.tile_pool(name="xpool", bufs=1))
        hpool = ctx.enter_context(tc.tile_pool(name="hpool", bufs=2))
        opool = ctx.enter_context(tc.tile_pool(name="opool", bufs=4))
        shpool = ctx.enter_context(tc.tile_pool(name="shpool", bufs=1))
        pspool = ctx.enter_context(tc.tile_pool(name="pspool", bufs=1, space="PSUM"))

        ps_tags = [f"ps{i}" for i in range(8)]

        # ---- SBUF weight tiles: per-(set, k) so compute can start per-slice
        wsb = []
        for s in range(3):
            w1k = [wpool.tile([P, F], BF16, name=f"w1s{s}k{k}") for k in range(KD)]
            w3k = [wpool.tile([P, F], BF16, name=f"w3s{s}k{k}") for k in range(KD)]
            w2k = [wpool.tile([P, D], BF16, name=f"w2s{s}k{k}") for k in range(KF)]
            wsb.append((w1k, w3k, w2k))

        # x: first-chunk per-k tiles + shared tile + expert remainder
        x0k = [xpool.tile([P, NT], BF16, name=f"x0k{k}") for k in range(KD)]
        xsh = xpool.tile([P, KD, SHT], BF16, name="xsh")
        xrest = (xpool.tile([P, KD, R - n0], BF16, name="xrest")
                 if R > n0 else None)

        shacc = shpool.tile([P, KD, SHT], F32, name="shacc")

        # ---- input DMA stream (sync queue), in consumption order
        for k in range(KD):
            nc.sync.dma_start(wsb[0][0][k][:], wd[0][0][:, k, :])
            nc.sync.dma_start(wsb[0][1][k][:], wd[0][1][:, k, :])
            nc.sync.dma_start(x0k[k][:, 0:n0], xall[:, k, 0:n0])
        for kf in range(KF):
            nc.sync.dma_start(wsb[0][2][kf][:], wd[0][2][:, kf, :])
        for k in range(KD):
            nc.sync.dma_start(wsb[1][0][k][:], wd[1][0][:, k, :])
            nc.sync.dma_start(wsb[1][1][k][:], wd[1][1][:, k, :])
        nc.sync.dma_start(xsh[:], xall[:, :, R:TOT])
        for kf in range(KF):
            nc.sync.dma_start(wsb[1][2][kf][:], wd[1][2][:, kf, :])
        for k in range(KD):
            nc.sync.dma_start(wsb[2][0][k][:], wd[2][0][:, k, :])
            nc.sync.dma_start(wsb[2][1][k][:], wd[2][1][:, k, :])
        for kf in range(KF):
            nc.sync.dma_start(wsb[2][2][kf][:], wd[2][2][:, kf, :])
        if xrest is not None:
            nc.sync.dma_start(xrest[:], xall[:, :, n0:R])

        def xsrc(ci, mode, k, c0, n):
            if mode != "e":
                return xsh[:, k, c0:c0 + n]
            if ci == 0:
                return x0k[k][:, 0:n]
            return xrest[:, k, c0 - n0:c0 - n0 + n]

        def ffn(ci, mode, s, c0, n):
            w1k, w3k, w2k = wsb[s]
            gc0 = c0 if mode == "e" else R + c0  # global output column offset
            h1s, h3s, hts = [None] * KF, [None] * KF, [None] * KF

            def act_mul(mf):
                a = hpool.tile([P, NT], F32, name="asb", tag="silu")
                nc.scalar.activation(a[:, 0:n], h1s[mf][:, 0:n], silu)
                ht = hpool.tile([P, NT], BF16, name="htsb", tag=f"ht{mf}")
                nc.vector.tensor_mul(ht[:, 0:n], a[:, 0:n], h3s[mf][:, 0:n])
                hts[mf] = ht

            if ci == 0:
                # k-outer: consume weight/x slices in DMA arrival order
                for mf in range(KF):
                    h1s[mf] = pspool.tile([P, NT], F32, name="h1ps", tag=ps_tags[2 * mf])
                    h3s[mf] = pspool.tile([P, NT], F32, name="h3ps", tag=ps_tags[2 * mf + 1])
                for k in range(KD):
                    xs = xsrc(ci, mode, k, c0, n)
                    st, sp = (k == 0), (k == KD - 1)
                    for mf in range(KF):
                        nc.tensor.matmul(h1s[mf][:, 0:n], w1k[k][:, mf * P:(mf + 1) * P],
                                         xs, start=st, stop=sp)
                        nc.tensor.matmul(h3s[mf][:, 0:n], w3k[k][:, mf * P:(mf + 1) * P],
                                         xs, start=st, stop=sp)
                for mf in range(KF):
                    act_mul(mf)
            else:
                # mf-outer: act/mul of pair mf overlaps matmuls of pair mf+1
                for mf in range(KF):
                    h1s[mf] = pspool.tile([P, NT], F32, name="h1ps", tag=ps_tags[2 * mf])
                    for k in range(KD):
                        nc.tensor.matmul(h1s[mf][:, 0:n], w1k[k][:, mf * P:(mf + 1) * P],
                                         xsrc(ci, mode, k, c0, n), start=(k == 0), stop=(k == KD - 1))
                    h3s[mf] = pspool.tile([P, NT], F32, name="h3ps", tag=ps_tags[2 * mf + 1])
                    for k in range(KD):
                        nc.tensor.matmul(h3s[mf][:, 0:n], w3k[k][:, mf * P:(mf + 1) * P],
                                         xsrc(ci, mode, k, c0, n), start=(k == 0), stop=(k == KD - 1))
                    act_mul(mf)

            # down-phase, kf-outer: PE never waits on the last ht
            yps = []
            for md in range(KD):
                yps.append(pspool.tile([P, NT], F32, name="yps", tag=ps_tags[md]))
            for kf in range(KF):
                st, sp = (kf == 0), (kf == KF - 1)
                for md in range(KD):
                    nc.tensor.matmul(yps[md][:, 0:n], w2k[kf][:, md * P:(md + 1) * P],
                                     hts[kf][:, 0:n], start=st, stop=sp)

            # PSUM->SBUF evacuation spread over three engines so bank reuse
            # (next chunk's up-phase) never waits on a single engine's queue
            copy_eng = [nc.vector.tensor_copy, nc.scalar.copy]
            if mode == "e":
                ysb = opool.tile([P, KD, NT], F32, name="ysb", tag="ysb")
                for md in range(KD):
                    copy_eng[md % 2](ysb[:, md, 0:n], yps[md][:, 0:n])
                nc.gpsimd.dma_start(yallr[:, :, gc0:gc0 + n], ysb[:, :, 0:n])
            elif mode == "s0":
                for md in range(KD):
                    copy_eng[md % 2](shacc[:, md, c0:c0 + n], yps[md][:, 0:n])
            else:  # s1: accumulate and emit one DMA
                for md in range(KD):
                    nc.vector.tensor_add(shacc[:, md, c0:c0 + n],
                                         shacc[:, md, c0:c0 + n], yps[md][:, 0:n])
                nc.gpsimd.dma_start(yallr[:, :, gc0:gc0 + n],
                                    shacc[:, :, c0:c0 + n])

        for ci, (mode, s, c0, n) in enumerate(chunks):
            ffn(ci, mode, s, c0, n)


def _get_nc(R):
    key = ("nc", R)
    if key not in _CACHE:
        nc = bacc.Bacc("TRN2", target_bir_lowering=False, debug=False,
                       num_devices=N_CORES)
        _emit(nc, R)
        nc.compile()
        _CACHE[key] = nc
    return _CACHE[key]


def _gate_numpy(x2d, gate_w, gate_bias):
    """Replicates reference _moe_gate in float64 (routing-stable)."""
    xl = x2d.astype(np.float64)
    logits = xl @ gate_w.astype(np.float64).T
    scores = 1.0 / (1.0 + np.exp(-logits))
    sc = scores + gate_bias.astype(np.float64)[None, :]
    grp = sc.reshape(T, N_GROUP, E // N_GROUP)
    group_scores = np.sort(grp, axis=-1)[:, :, -2:].sum(-1)
    gidx = np.argsort(-group_scores, axis=-1, kind="stable")[:, :TOPK_GROUP]
    gmask = np.zeros((T, N_GROUP), bool)
    gmask[np.arange(T)[:, None], gidx] = True
    smask = np.repeat(gmask, E // N_GROUP, axis=1)
    tmp = np.where(smask, sc, 0.0)
    tidx = np.argsort(-tmp, axis=-1, kind="stable")[:, :TOP_K]
    tw = np.take_along_axis(scores, tidx, axis=1)
    tw = tw / (tw.sum(-1, keepdims=True) + 1e-20)
    return tidx, (tw * SCALE).astype(np.float32)


def _ffn_host(x, w1e, w2e, w3e):
    """Host fallback for capacity-overflow tokens (pathological skew only)."""
    h = x @ w1e.T
    h = (h / (1.0 + np.exp(-h))) * (x @ w3e.T)
    return h @ w2e.T


def _ikp(mat, kt):
    """[kt*P, X] -> [P, kt, X] bf16 (k-interleaved, partition-major)."""
    return np.asarray(mat).reshape(kt, P, -1).transpose(1, 0, 2).astype(BF16_NP)


def kernel(hidden_states, gate_w, gate_bias, w1, w2, w3,
           shared_gate_w, shared_up_w, shared_down_w):
    hidden_states = np.ascontiguousarray(np.asarray(hidden_states, np.float32))
    gate_w = np.asarray(gate_w, np.float32)
    gate_bias = np.asarray(gate_bias, np.float32)
    w1 = np.asarray(w1, np.float32)
    w2 = np.asarray(w2, np.float32)
    w3 = np.asarray(w3, np.float32)
    shared_gate_w = np.asarray(shared_gate_w, np.float32)
    shared_up_w = np.asarray(shared_up_w, np.float32)
    shared_down_w = np.asarray(shared_down_w, np.float32)

    x2d = hidden_states.reshape(T, D)
    tidx, tw = _gate_numpy(x2d, gate_w, gate_bias)

    counts = np.bincount(tidx.ravel(), minlength=E)
    R = int(min(-(-counts.max() // 32) * 32, R_MAX))
    R = max(R, 32)
    TOT = R + SHT

    # weight sets: expert e -> set0 of core e; shared halves -> sets 1,2
    wsets_e = []
    for e in range(E):
        wsets_e.append((_ikp(w1[e].T, KD), _ikp(w3[e].T, KD), _ikp(w2[e].T, KF)))
    wsets_sh = []
    for h in range(2):
        hf = slice(h * F, (h + 1) * F)
        wsets_sh.append((_ikp(shared_gate_w[hf].T, KD),
                         _ikp(shared_up_w[hf].T, KD),
                         _ikp(shared_down_w[:, hf].T, KF)))

    x2dT = np.ascontiguousarray(x2d.T)  # [D, T]
    in_maps = []
    idx_list, wt_list, n_list, overflow = [], [], [], []
    for e in range(E):
        rows, slots = np.nonzero(tidx == e)
        n = len(rows)
        if n > R:
            overflow.append((e, rows[R:], slots[R:]))
            rows, slots = rows[:R], slots[:R]
            n = R
        idx_list.append(rows)
        wt_list.append(tw[rows, slots])
        n_list.append(n)
        xcols = np.zeros((D, TOT), np.float32)
        xcols[:, :n] = x2dT[:, rows]
        xcols[:, R:] = x2dT[:, e * SHT:(e + 1) * SHT]
        im = {"xall": _ikp(xcols, KD)}
        for s, ws in ((0, wsets_e[e]), (1, wsets_sh[0]), (2, wsets_sh[1])):
            im[f"w1_{s}"], im[f"w3_{s}"], im[f"w2_{s}"] = ws
        in_maps.append(im)

    nc = _get_nc(R)
    res = bass_utils.run_bass_kernel_spmd(
        nc, in_maps, core_ids=list(range(N_CORES))
    )
    _CACHE["last_res"] = res

    y = np.zeros((T, D), np.float32)
    for e in range(E):
        n = n_list[e]
        out = res.results[e]["yall"]
        if n:
            y[idx_list[e]] += wt_list[e][:, None] * out[:, :n].T
        y[e * SHT:(e + 1) * SHT] += out[:, R:].T
    for e, rows, slots in overflow:
        y[rows] += tw[rows, slots][:, None] * _ffn_host(x2d[rows], w1[e], w2[e], w3[e])

    return y.reshape(B, S, D)


# revision 12
# speedup vs baseline: 1.1107x; 1.1107x over previous
"""Expert-parallel MoE (Kimi/DeepSeek-V3 sparse block) on 8 trn2 NeuronCores.

Strategy (v5):
  - Host computes the sigmoid gate + group-limited top-2 routing in float64
    and gathers each expert's tokens into a transposed, k-interleaved batch.
  - Capacity R = align32(max expert load) is chosen from the *actual*
    routing, so per-core work is R + 2*512 shared token-instances instead of
    a fixed 1536 + 1024 — the program is compiled per capacity and cached.
  - All matmul operands are bf16 (fp32 PSUM accumulate): halves HBM->SBUF
    traffic; error ~4e-3 vs the 2e-2 gate.
  - Every DMA moves >=2KB contiguous per partition (the DMA-efficiency
    knee): first-chunk weights/x stream at k-pair granularity so the first
    matmul starts ~2us after the queue opens; later sets move whole-matrix.
  - Chunk order E0, S0, S1, E1, ..., Er: weights/x for S0 arrive during E0;
    the small expert remainder lands last so the post-matmul tail is short.
  - Up-phase mf-outer (k-outer for the streaming first chunk), down-phase
    kf-outer; PSUM->SBUF evacuation alternates vector/scalar engines.
  - Per-chunk contiguous output tensors ([P, KD, n], 16KB lines) written by
    one DMA on the gpsimd queue.
  - ~48 tiny warm-up matmuls run during the initial DMA wait so the PE's
    HAM clock gate is already 8/8 when real work starts.
"""

from contextlib import ExitStack

import numpy as np
import ml_dtypes

import concourse.bacc as bacc
import concourse.tile as tile
import concourse.mybir as mybir
from concourse import bass_utils

# --- model dims (hardcoded per problem spec) ---
B, S, D = 2, 2048, 1024
T = B * S                 # 4096 tokens
E, F = 8, 512             # routed experts / expert intermediate
SH = 1024                 # shared intermediate (= 2 halves of F)
TOP_K, N_GROUP, TOPK_GROUP = 2, 4, 2
SCALE = 2.5

N_CORES = 8
P = 128                   # SBUF partitions
KD = D // P               # 8 contraction tiles over D
KF = F // P               # 4 contraction tiles over F
NT = 512                  # max matmul moving free dim (one PSUM bank fp32)
SHT = T // N_CORES        # 512 shared-expert tokens per core
R_MAX = 4096              # capacity ceiling (SBUF); overflow -> host
N_WARM = 48               # HAM warm-up matmuls

F32 = mybir.dt.float32
BF16 = mybir.dt.bfloat16
BF16_NP = ml_dtypes.bfloat16

_CACHE: dict = {}


def _chunk_plan(R):
    """Chunks: (mode, wset, col_off, ncols).

    Order: first expert chunk (k-pair DMA streaming), then the two shared
    passes, then remaining expert chunks — the small expert remainder lands
    last so the post-matmul tail (copies + output DMA) is short."""
    echunks = []
    c = 0
    while c < R:
        n = min(NT, R - c)
        echunks.append(("e", 0, c, n))
        c += n
    schunks = []
    for mode, s in (("s0", 1), ("s1", 2)):
        c = 0
        while c < SHT:
            n = min(NT, SHT - c)
            schunks.append((mode, s, c, n))
            c += n
    return tuple(echunks[:1] + schunks + echunks[1:])


def _emit(nc, R):
    chunks = _chunk_plan(R)
    n0 = chunks[0][3]  # first chunk size

    xd0 = nc.dram_tensor("xd0", [P, KD, n0], BF16, kind="ExternalInput").ap()
    xshd = nc.dram_tensor("xshd", [P, KD, SHT], BF16, kind="ExternalInput").ap()
    xrd = (nc.dram_tensor("xrd", [P, KD, R - n0], BF16, kind="ExternalInput").ap()
           if R > n0 else None)
    wd = []
    for s in range(3):
        w1 = nc.dram_tensor(f"w1_{s}", [P, KD, F], BF16, kind="ExternalInput").ap()
        w3 = nc.dram_tensor(f"w3_{s}", [P, KD, F], BF16, kind="ExternalInput").ap()
        w2 = nc.dram_tensor(f"w2_{s}", [P, KF, D], BF16, kind="ExternalInput").ap()
        wd.append((w1, w3, w2))
    youts = []
    for j, (mode, s, c0, n) in enumerate(chunks):
        youts.append(nc.dram_tensor(f"y{j}", [P, KD, n], F32, kind="ExternalOutput").ap()
                     if mode != "s0" else None)

    silu = mybir.ActivationFunctionType.Silu

    with tile.TileContext(nc) as tc, ExitStack() as ctx:
        wpool = ctx.enter_context(tc.tile_pool(name="wpool", bufs=1))
        xpool = ctx.enter_context(tc.tile_pool(name="xpool", bufs=1))
        hpool = ctx.enter_context(tc.tile_pool(name="hpool", bufs=2))
        opool = ctx.enter_context(tc.tile_pool(name="opool", bufs=2))
        shpool = ctx.enter_context(tc.tile_pool(name="shpool", bufs=1))
        pspool = ctx.enter_context(tc.tile_pool(name="pspool", bufs=1, space="PSUM"))

        ps_tags = [f"ps{i}" for i in range(8)]

        # ---- SBUF weight tiles: set0 at k-pair granularity, others whole
        w1p = [wpool.tile([P, 2, F], BF16, name=f"w1p{j}") for j in range(KD // 2)]
        w3p = [wpool.tile([P, 2, F], BF16, name=f"w3p{j}") for j in range(KD // 2)]
        w2t = [wpool.tile([P, KF, D], BF16, name=f"w2t{s}") for s in range(3)]
        w1s = [None] + [wpool.tile([P, KD, F], BF16, name=f"w1s{s}") for s in (1, 2)]
        w3s = [None] + [wpool.tile([P, KD, F], BF16, name=f"w3s{s}") for s in (1, 2)]

        x0p = [xpool.tile([P, 2, n0], BF16, name=f"x0p{j}") for j in range(KD // 2)]
        xsh = xpool.tile([P, KD, SHT], BF16, name="xsh")
        xrest = (xpool.tile([P, KD, R - n0], BF16, name="xrest")
                 if R > n0 else None)

        shacc = shpool.tile([P, KD, SHT], F32, name="shacc")

        # ---- HAM warm-up: tiny matmuls on a zeroed tile during DMA wait
        wu = xpool.tile([P, 64], BF16, name="wu")
        nc.vector.memset(wu[:], 0)
        wups = pspool.tile([P, NT], F32, name="wups", tag=ps_tags[7])
        for _ in range(N_WARM):
            nc.tensor.matmul(wups[0:64, 0:64], wu[:, 0:64], wu[:, 0:64],
                             start=True, stop=True)

        # ---- input DMA stream (sync queue), in consumption order
        for j in range(KD // 2):
            kk = slice(2 * j, 2 * j + 2)
            nc.sync.dma_start(w1p[j][:], wd[0][0][:, kk, :])
            nc.sync.dma_start(w3p[j][:], wd[0][1][:, kk, :])
            nc.sync.dma_start(x0p[j][:], xd0[:, kk, :])
        nc.sync.dma_start(w2t[0][:], wd[0][2][:])
        nc.sync.dma_start(w1s[1][:], wd[1][0][:])
        nc.sync.dma_start(w3s[1][:], wd[1][1][:])
        nc.sync.dma_start(xsh[:], xshd[:])
        nc.sync.dma_start(w2t[1][:], wd[1][2][:])
        nc.sync.dma_start(w1s[2][:], wd[2][0][:])
        nc.sync.dma_start(w3s[2][:], wd[2][1][:])
        nc.sync.dma_start(w2t[2][:], wd[2][2][:])
        if xrest is not None:
            nc.sync.dma_start(xrest[:], xrd[:])

        def wsl(which, s, k, blk):
            if s == 0:
                t = (w1p if which == 1 else w3p)[k // 2]
                return t[:, k % 2, blk * P:(blk + 1) * P]
            t = (w1s if which == 1 else w3s)[s]
            return t[:, k, blk * P:(blk + 1) * P]

        def xsrc(ci, mode, k, c0, n):
            if mode != "e":
                return xsh[:, k, c0:c0 + n]
            if ci == 0:
                return x0p[k // 2][:, k % 2, 0:n]
            return xrest[:, k, c0 - n0:c0 - n0 + n]

        def ffn(ci, mode, s, c0, n):
            yj = youts[ci]
            h1s, h3s, hts = [None] * KF, [None] * KF, [None] * KF

            def act_mul(mf):
                a = hpool.tile([P, NT], F32, name="asb", tag="silu")
                nc.scalar.activation(a[:, 0:n], h1s[mf][:, 0:n], silu)
                ht = hpool.tile([P, NT], BF16, name="htsb", tag=f"ht{mf}")
                nc.vector.tensor_mul(ht[:, 0:n], a[:, 0:n], h3s[mf][:, 0:n])
                hts[mf] = ht

            if ci == 0:
                # k-outer: consume weight/x k-pairs in DMA arrival order
                for mf in range(KF):
                    h1s[mf] = pspool.tile([P, NT], F32, name="h1ps", tag=ps_tags[2 * mf])
                    h3s[mf] = pspool.tile([P, NT], F32, name="h3ps", tag=ps_tags[2 * mf + 1])
                for k in range(KD):
                    xs = xsrc(ci, mode, k, c0, n)
                    st, sp = (k == 0), (k == KD - 1)
                    for mf in range(KF):
                        nc.tensor.matmul(h1s[mf][:, 0:n], wsl(1, s, k, mf),
                                         xs, start=st, stop=sp)
                        nc.tensor.matmul(h3s[mf][:, 0:n], wsl(3, s, k, mf),
                                         xs, start=st, stop=sp)
                for mf in range(KF):
                    act_mul(mf)
            else:
                # mf-outer: act/mul of pair mf overlaps matmuls of pair mf+1
                for mf in range(KF):
                    h1s[mf] = pspool.tile([P, NT], F32, name="h1ps", tag=ps_tags[2 * mf])
                    for k in range(KD):
                        nc.tensor.matmul(h1s[mf][:, 0:n], wsl(1, s, k, mf),
                                         xsrc(ci, mode, k, c0, n),
                                         start=(k == 0), stop=(k == KD - 1))
                    h3s[mf] = pspool.tile([P, NT], F32, name="h3ps", tag=ps_tags[2 * mf + 1])
                    for k in range(KD):
                        nc.tensor.matmul(h3s[mf][:, 0:n], wsl(3, s, k, mf),
                                         xsrc(ci, mode, k, c0, n),
                                         start=(k == 0), stop=(k == KD - 1))
                    act_mul(mf)

            # down-phase, kf-outer: PE never waits on the last ht
            yps = []
            for md in range(KD):
                yps.append(pspool.tile([P, NT], F32, name="yps", tag=ps_tags[md]))
            for kf in range(KF):
                st, sp = (kf == 0), (kf == KF - 1)
                for md in range(KD):
                    nc.tensor.matmul(yps[md][:, 0:n], w2t[s][:, kf, md * P:(md + 1) * P],
                                     hts[kf][:, 0:n], start=st, stop=sp)

            # PSUM->SBUF evacuation alternating engines; one DMA per chunk
            copy_eng = [nc.vector.tensor_copy, nc.scalar.copy]
            if mode == "e":
                ysb = opool.tile([P, KD, NT], F32, name="ysb", tag="ysb")
                for md in range(KD):
                    copy_eng[md % 2](ysb[:, md, 0:n], yps[md][:, 0:n])
                nc.gpsimd.dma_start(yj[:], ysb[:, :, 0:n])
            elif mode == "s0":
                for md in range(KD):
                    copy_eng[md % 2](shacc[:, md, c0:c0 + n], yps[md][:, 0:n])
            else:  # s1: accumulate and emit one DMA
                for md in range(KD):
                    nc.vector.tensor_add(shacc[:, md, c0:c0 + n],
                                         shacc[:, md, c0:c0 + n], yps[md][:, 0:n])
                nc.gpsimd.dma_start(yj[:], shacc[:, :, c0:c0 + n])

        for ci, (mode, s, c0, n) in enumerate(chunks):
            ffn(ci, mode, s, c0, n)


def _get_nc(R):
    key = ("nc", R)
    if key not in _CACHE:
        nc = bacc.Bacc("TRN2", target_bir_lowering=False, debug=False,
                       num_devices=N_CORES)
        _emit(nc, R)
        nc.compile()
        _CACHE[key] = nc
    return _CACHE[key]


def _gate_numpy(x2d, gate_w, gate_bias):
    """Replicates reference _moe_gate in float64 (routing-stable)."""
    xl = x2d.astype(np.float64)
    logits = xl @ gate_w.astype(np.float64).T
    scores = 1.0 / (1.0 + np.exp(-logits))
    sc = scores + gate_bias.astype(np.float64)[None, :]
    grp = sc.reshape(T, N_GROUP, E // N_GROUP)
    group_scores = np.sort(grp, axis=-1)[:, :, -2:].sum(-1)
    gidx = np.argsort(-group_scores, axis=-1, kind="stable")[:, :TOPK_GROUP]
    gmask = np.zeros((T, N_GROUP), bool)
    gmask[np.arange(T)[:, None], gidx] = True
    smask = np.repeat(gmask, E // N_GROUP, axis=1)
    tmp = np.where(smask, sc, 0.0)
    tidx = np.argsort(-tmp, axis=-1, kind="stable")[:, :TOP_K]
    tw = np.take_along_axis(scores, tidx, axis=1)
    tw = tw / (tw.sum(-1, keepdims=True) + 1e-20)
    return tidx, (tw * SCALE).astype(np.float32)


def _ffn_host(x, w1e, w2e, w3e):
    """Host fallback for capacity-overflow tokens (pathological skew only)."""
    h = x @ w1e.T
    h = (h / (1.0 + np.exp(-h))) * (x @ w3e.T)
    return h @ w2e.T


def _ikp(mat, kt):
    """[kt*P, X] -> [P, kt, X] bf16 (k-interleaved, partition-major)."""
    return np.asarray(mat).reshape(kt, P, -1).transpose(1, 0, 2).astype(BF16_NP)


def kernel(hidden_states, gate_w, gate_bias, w1, w2, w3,
           shared_gate_w, shared_up_w, shared_down_w):
    hidden_states = np.ascontiguousarray(np.asarray(hidden_states, np.float32))
    gate_w = np.asarray(gate_w, np.float32)
    gate_bias = np.asarray(gate_bias, np.float32)
    w1 = np.asarray(w1, np.float32)
    w2 = np.asarray(w2, np.float32)
    w3 = np.asarray(w3, np.float32)
    shared_gate_w = np.asarray(shared_gate_w, np.float32)
    shared_up_w = np.asarray(shared_up_w, np.float32)
    shared_down_w = np.asarray(shared_down_w, np.float32)

    x2d = hidden_states.reshape(T, D)
    tidx, tw = _gate_numpy(x2d, gate_w, gate_bias)

    counts = np.bincount(tidx.ravel(), minlength=E)
    R = int(min(-(-counts.max() // 32) * 32, R_MAX))
    R = max(R, 32)
    chunks = _chunk_plan(R)
    n0 = chunks[0][3]

    # weight sets: expert e -> set0 of core e; shared halves -> sets 1,2
    wsets_e = []
    for e in range(E):
        wsets_e.append((_ikp(w1[e].T, KD), _ikp(w3[e].T, KD), _ikp(w2[e].T, KF)))
    wsets_sh = []
    for h in range(2):
        hf = slice(h * F, (h + 1) * F)
        wsets_sh.append((_ikp(shared_gate_w[hf].T, KD),
                         _ikp(shared_up_w[hf].T, KD),
                         _ikp(shared_down_w[:, hf].T, KF)))

    x2dT = np.ascontiguousarray(x2d.T)  # [D, T]
    in_maps = []
    idx_list, wt_list, n_list, overflow = [], [], [], []
    for e in range(E):
        rows, slots = np.nonzero(tidx == e)
        n = len(rows)
        if n > R:
            overflow.append((e, rows[R:], slots[R:]))
            rows, slots = rows[:R], slots[:R]
            n = R
        idx_list.append(rows)
        wt_list.append(tw[rows, slots])
        n_list.append(n)
        xe = np.zeros((D, R), np.float32)
        xe[:, :n] = x2dT[:, rows]
        im = {
            "xd0": _ikp(xe[:, :n0], KD),
            "xshd": _ikp(x2dT[:, e * SHT:(e + 1) * SHT], KD),
        }
        if R > n0:
            im["xrd"] = _ikp(xe[:, n0:], KD)
        for s, ws in ((0, wsets_e[e]), (1, wsets_sh[0]), (2, wsets_sh[1])):
            im[f"w1_{s}"], im[f"w3_{s}"], im[f"w2_{s}"] = ws
        in_maps.append(im)

    nc = _get_nc(R)
    res = bass_utils.run_bass_kernel_spmd(
        nc, in_maps, core_ids=list(range(N_CORES))
    )
    _CACHE["last_res"] = res

    y = np.zeros((T, D), np.float32)
    for e in range(E):
        n = n_list[e]
        rows = idx_list[e]
        wts = wt_list[e]
        out = res.results[e]
        for j, (mode, s, c0, nj) in enumerate(chunks):
            if mode == "s0":
                continue
            # y{j} is [P, KD, nj]; row d = md*P + p
            blk = out[f"y{j}"].transpose(1, 0, 2).reshape(D, nj)
            if mode == "e":
                lo, hi = c0, min(c0 + nj, n)
                if hi > lo:
                    cols = blk[:, 0:hi - lo].T  # [tokens, D]
                    y[rows[lo:hi]] += wts[lo:hi, None] * cols
            else:  # s1: shared output for token slice
                sl = slice(e * SHT + c0, e * SHT + c0 + nj)
                y[sl] += blk.T
    for e, rows, slots in overflow:
        y[rows] += tw[rows, slots][:, None] * _ffn_host(x2d[rows], w1[e], w2[e], w3[e])

    return y.reshape(B, S, D)


# revision 17
# speedup vs baseline: 1.1500x; 1.0354x over previous
"""Expert-parallel MoE (Kimi/DeepSeek-V3 sparse block) on 8 trn2 NeuronCores.

Strategy (v5):
  - Host computes the sigmoid gate + group-limited top-2 routing in float64
    and gathers each expert's tokens into a transposed, k-interleaved batch.
  - Capacity R = align32(max expert load) is chosen from the *actual*
    routing, so per-core work is R + 2*512 shared token-instances instead of
    a fixed 1536 + 1024 — the program is compiled per capacity and cached.
  - All matmul operands are bf16 (fp32 PSUM accumulate): halves HBM->SBUF
    traffic; error ~4e-3 vs the 2e-2 gate.
  - Every DMA moves >=2KB contiguous per partition (the DMA-efficiency
    knee): first-chunk weights/x stream at k-pair granularity so the first
    matmul starts ~2us after the queue opens; later sets move whole-matrix.
  - Chunk order E0, S0, S1, E1, ..., Er: weights/x for S0 arrive during E0;
    the small expert remainder lands last so the post-matmul tail is short.
  - Up-phase mf-outer (k-outer for the streaming first chunk), down-phase
    kf-outer; PSUM->SBUF evacuation alternates vector/scalar engines.
  - Per-chunk contiguous output tensors ([P, KD, n], 16KB lines) written by
    one DMA on the gpsimd queue.
  - ~48 tiny warm-up matmuls run during the initial DMA wait so the PE's
    HAM clock gate is already 8/8 when real work starts.
"""

from contextlib import ExitStack

import numpy as np
import ml_dtypes

import concourse.bacc as bacc
import concourse.tile as tile
import concourse.mybir as mybir
from concourse import bass_utils

# --- model dims (hardcoded per problem spec) ---
B, S, D = 2, 2048, 1024
T = B * S                 # 4096 tokens
E, F = 8, 512             # routed experts / expert intermediate
SH = 1024                 # shared intermediate (= 2 halves of F)
TOP_K, N_GROUP, TOPK_GROUP = 2, 4, 2
SCALE = 2.5

N_CORES = 8
P = 128                   # SBUF partitions
KD = D // P               # 8 contraction tiles over D
KF = F // P               # 4 contraction tiles over F
NT = 512                  # max matmul moving free dim (one PSUM bank fp32)
SHT = T // N_CORES        # 512 shared-expert tokens per core
R_MAX = 4096              # capacity ceiling (SBUF); overflow -> host
N_WARM = 40                # HAM warm-up matmuls (cover the DMA head)

F32 = mybir.dt.float32
BF16 = mybir.dt.bfloat16
BF16_NP = ml_dtypes.bfloat16

_CACHE: dict = {}


def _chunk_plan(R):
    """Chunks: (mode, wset, col_off, ncols).

    Order: first expert chunk (k-pair DMA streaming), then the two shared
    passes, then remaining expert chunks — the small expert remainder lands
    last so the post-matmul tail (copies + output DMA) is short."""
    echunks = []
    c = 0
    while c < R:
        n = min(NT, R - c)
        echunks.append(("e", 0, c, n))
        c += n
    schunks = []
    for mode, s in (("s0", 1), ("s1", 2)):
        c = 0
        while c < SHT:
            n = min(NT, SHT - c)
            schunks.append((mode, s, c, n))
            c += n
    return tuple(echunks[:1] + schunks + echunks[1:])


def _emit(nc, R):
    chunks = _chunk_plan(R)
    n0 = chunks[0][3]  # first chunk size
    W0 = 2 * F + n0    # packed set0 row: w1 | w3 | x0 per k

    s0d = nc.dram_tensor("s0pack", [P, KD, W0], BF16, kind="ExternalInput").ap()
    xshd = nc.dram_tensor("xshd", [P, KD, SHT], BF16, kind="ExternalInput").ap()
    xrd = (nc.dram_tensor("xrd", [P, KD, R - n0], BF16, kind="ExternalInput").ap()
           if R > n0 else None)
    wd = [None]
    for s in (1, 2):
        w1 = nc.dram_tensor(f"w1_{s}", [P, KD, F], BF16, kind="ExternalInput").ap()
        w3 = nc.dram_tensor(f"w3_{s}", [P, KD, F], BF16, kind="ExternalInput").ap()
        wd.append((w1, w3))
    w2d = [nc.dram_tensor(f"w2_{s}", [P, KF, D], BF16, kind="ExternalInput").ap()
           for s in range(3)]
    youts = []
    for j, (mode, s, c0, n) in enumerate(chunks):
        youts.append(nc.dram_tensor(f"y{j}", [P, KD, n], F32, kind="ExternalOutput").ap()
                     if mode != "s0" else None)

    silu = mybir.ActivationFunctionType.Silu

    with tile.TileContext(nc) as tc, ExitStack() as ctx:
        wpool = ctx.enter_context(tc.tile_pool(name="wpool", bufs=1))
        xpool = ctx.enter_context(tc.tile_pool(name="xpool", bufs=1))
        hpool = ctx.enter_context(tc.tile_pool(name="hpool", bufs=2))
        opool = ctx.enter_context(tc.tile_pool(name="opool", bufs=2))
        shpool = ctx.enter_context(tc.tile_pool(name="shpool", bufs=1))
        pspool = ctx.enter_context(tc.tile_pool(name="pspool", bufs=1, space="PSUM"))

        ps_tags = [f"ps{i}" for i in range(8)]

        # ---- SBUF weight tiles: set0 packed at k-pair granularity
        s0p = [wpool.tile([P, 2, W0], BF16, name=f"s0p{j}") for j in range(KD // 2)]
        w2t = [wpool.tile([P, KF, D], BF16, name=f"w2t{s}") for s in range(3)]
        w1s = [None] + [wpool.tile([P, KD, F], BF16, name=f"w1s{s}") for s in (1, 2)]
        w3s = [None] + [wpool.tile([P, KD, F], BF16, name=f"w3s{s}") for s in (1, 2)]

        xsh = xpool.tile([P, KD, SHT], BF16, name="xsh")
        xrest = (xpool.tile([P, KD, R - n0], BF16, name="xrest")
                 if R > n0 else None)

        shacc = shpool.tile([P, KD, SHT], F32, name="shacc")

        # ---- HAM warm-up: tiny matmuls on a zeroed tile during DMA wait
        wu = xpool.tile([P, 64], BF16, name="wu")
        nc.vector.memset(wu[:], 0)
        wups = pspool.tile([P, NT], F32, name="wups", tag=ps_tags[7])
        for _ in range(N_WARM):
            nc.tensor.matmul(wups[0:64, 0:64], wu[:, 0:64], wu[:, 0:64],
                             start=True, stop=True)

        # ---- input DMA stream (sync queue), in consumption order
        for j in range(KD // 2):
            nc.sync.dma_start(s0p[j][:], s0d[:, 2 * j:2 * j + 2, :])
        nc.sync.dma_start(w2t[0][:], w2d[0][:])
        nc.sync.dma_start(w1s[1][:], wd[1][0][:])
        nc.sync.dma_start(w3s[1][:], wd[1][1][:])
        nc.sync.dma_start(xsh[:], xshd[:])
        nc.sync.dma_start(w2t[1][:], w2d[1][:])
        nc.sync.dma_start(w1s[2][:], wd[2][0][:])
        nc.sync.dma_start(w3s[2][:], wd[2][1][:])
        nc.sync.dma_start(w2t[2][:], w2d[2][:])
        if xrest is not None:
            nc.sync.dma_start(xrest[:], xrd[:])

        def wsl(which, s, k, blk):
            if s == 0:
                off = 0 if which == 1 else F
                return s0p[k // 2][:, k % 2, off + blk * P:off + (blk + 1) * P]
            t = (w1s if which == 1 else w3s)[s]
            return t[:, k, blk * P:(blk + 1) * P]

        def xsrc(ci, mode, k, c0, n):
            if mode != "e":
                return xsh[:, k, c0:c0 + n]
            if ci == 0:
                return s0p[k // 2][:, k % 2, 2 * F:2 * F + n]
            return xrest[:, k, c0 - n0:c0 - n0 + n]

        def ffn(ci, mode, s, c0, n):
            yj = youts[ci]
            h1s, h3s, hts = [None] * KF, [None] * KF, [None] * KF

            def act_mul(mf):
                a = hpool.tile([P, NT], F32, name="asb", tag="silu")
                nc.scalar.activation(a[:, 0:n], h1s[mf][:, 0:n], silu)
                ht = hpool.tile([P, NT], BF16, name="htsb", tag=f"ht{mf}")
                nc.vector.tensor_mul(ht[:, 0:n], a[:, 0:n], h3s[mf][:, 0:n])
                hts[mf] = ht

            if ci == 0:
                # k-outer: consume weight/x k-pairs in DMA arrival order
                for mf in range(KF):
                    h1s[mf] = pspool.tile([P, NT], F32, name="h1ps", tag=ps_tags[2 * mf])
                    h3s[mf] = pspool.tile([P, NT], F32, name="h3ps", tag=ps_tags[2 * mf + 1])
                for k in range(KD):
                    xs = xsrc(ci, mode, k, c0, n)
                    st, sp = (k == 0), (k == KD - 1)
                    for mf in range(KF):
                        nc.tensor.matmul(h1s[mf][:, 0:n], wsl(1, s, k, mf),
                                         xs, start=st, stop=sp)
                        nc.tensor.matmul(h3s[mf][:, 0:n], wsl(3, s, k, mf),
                                         xs, start=st, stop=sp)
                for mf in range(KF):
                    act_mul(mf)
            else:
                # mf-outer: act/mul of pair mf overlaps matmuls of pair mf+1
                for mf in range(KF):
                    h1s[mf] = pspool.tile([P, NT], F32, name="h1ps", tag=ps_tags[2 * mf])
                    for k in range(KD):
                        nc.tensor.matmul(h1s[mf][:, 0:n], wsl(1, s, k, mf),
                                         xsrc(ci, mode, k, c0, n),
                                         start=(k == 0), stop=(k == KD - 1))
                    h3s[mf] = pspool.tile([P, NT], F32, name="h3ps", tag=ps_tags[2 * mf + 1])
                    for k in range(KD):
                        nc.tensor.matmul(h3s[mf][:, 0:n], wsl(3, s, k, mf),
                                         xsrc(ci, mode, k, c0, n),
                                         start=(k == 0), stop=(k == KD - 1))
                    act_mul(mf)

            # down-phase in two md-halves (kf-outer inside each) so the
            # first half's outputs flush while the second half computes
            copy_eng = [nc.vector.tensor_copy, nc.scalar.copy]
            ysb = (opool.tile([P, KD, NT], F32, name="ysb", tag="ysb")
                   if mode == "e" else None)
            H = KD // 2
            for half in range(2):
                mds = range(half * H, (half + 1) * H)
                yps = {md: pspool.tile([P, NT], F32, name="yps", tag=ps_tags[md])
                       for md in mds}
                for kf in range(KF):
                    st, sp = (kf == 0), (kf == KF - 1)
                    for md in mds:
                        nc.tensor.matmul(yps[md][:, 0:n],
                                         w2t[s][:, kf, md * P:(md + 1) * P],
                                         hts[kf][:, 0:n], start=st, stop=sp)
                hsl = slice(half * H, (half + 1) * H)
                if mode == "e":
                    for md in mds:
                        copy_eng[md % 2](ysb[:, md, 0:n], yps[md][:, 0:n])
                    nc.gpsimd.dma_start(yj[:, hsl, :], ysb[:, hsl, 0:n])
                elif mode == "s0":
                    for md in mds:
                        copy_eng[md % 2](shacc[:, md, c0:c0 + n], yps[md][:, 0:n])
                else:  # s1: accumulate and emit
                    for md in mds:
                        nc.vector.tensor_add(shacc[:, md, c0:c0 + n],
                                             shacc[:, md, c0:c0 + n], yps[md][:, 0:n])
                    nc.gpsimd.dma_start(yj[:, hsl, :], shacc[:, hsl, c0:c0 + n])

        for ci, (mode, s, c0, n) in enumerate(chunks):
            ffn(ci, mode, s, c0, n)


def _get_nc(R):
    key = ("nc", R)
    if key not in _CACHE:
        nc = bacc.Bacc("TRN2", target_bir_lowering=False, debug=False,
                       num_devices=N_CORES)
        _emit(nc, R)
        nc.compile()
        _CACHE[key] = nc
    return _CACHE[key]


def _gate_numpy(x2d, gate_w, gate_bias):
    """Replicates reference _moe_gate in float64 (routing-stable)."""
    xl = x2d.astype(np.float64)
    logits = xl @ gate_w.astype(np.float64).T
    scores = 1.0 / (1.0 + np.exp(-logits))
    sc = scores + gate_bias.astype(np.float64)[None, :]
    grp = sc.reshape(T, N_GROUP, E // N_GROUP)
    group_scores = np.sort(grp, axis=-1)[:, :, -2:].sum(-1)
    gidx = np.argsort(-group_scores, axis=-1, kind="stable")[:, :TOPK_GROUP]
    gmask = np.zeros((T, N_GROUP), bool)
    gmask[np.arange(T)[:, None], gidx] = True
    smask = np.repeat(gmask, E // N_GROUP, axis=1)
    tmp = np.where(smask, sc, 0.0)
    tidx = np.argsort(-tmp, axis=-1, kind="stable")[:, :TOP_K]
    tw = np.take_along_axis(scores, tidx, axis=1)
    tw = tw / (tw.sum(-1, keepdims=True) + 1e-20)
    return tidx, (tw * SCALE).astype(np.float32)


def _ffn_host(x, w1e, w2e, w3e):
    """Host fallback for capacity-overflow tokens (pathological skew only)."""
    h = x @ w1e.T
    h = (h / (1.0 + np.exp(-h))) * (x @ w3e.T)
    return h @ w2e.T


def _ikp(mat, kt):
    """[kt*P, X] -> [P, kt, X] bf16 (k-interleaved, partition-major)."""
    return np.asarray(mat).reshape(kt, P, -1).transpose(1, 0, 2).astype(BF16_NP)


def kernel(hidden_states, gate_w, gate_bias, w1, w2, w3,
           shared_gate_w, shared_up_w, shared_down_w):
    hidden_states = np.ascontiguousarray(np.asarray(hidden_states, np.float32))
    gate_w = np.asarray(gate_w, np.float32)
    gate_bias = np.asarray(gate_bias, np.float32)
    w1 = np.asarray(w1, np.float32)
    w2 = np.asarray(w2, np.float32)
    w3 = np.asarray(w3, np.float32)
    shared_gate_w = np.asarray(shared_gate_w, np.float32)
    shared_up_w = np.asarray(shared_up_w, np.float32)
    shared_down_w = np.asarray(shared_down_w, np.float32)

    x2d = hidden_states.reshape(T, D)
    tidx, tw = _gate_numpy(x2d, gate_w, gate_bias)

    counts = np.bincount(tidx.ravel(), minlength=E)
    R = int(min(-(-counts.max() // 32) * 32, R_MAX))
    R = max(R, 32)
    chunks = _chunk_plan(R)
    n0 = chunks[0][3]

    # weight sets: expert e -> set0 of core e; shared halves -> sets 1,2
    wsets_e = []
    for e in range(E):
        wsets_e.append((_ikp(w1[e].T, KD), _ikp(w3[e].T, KD), _ikp(w2[e].T, KF)))
    wsets_sh = []
    for h in range(2):
        hf = slice(h * F, (h + 1) * F)
        wsets_sh.append((_ikp(shared_gate_w[hf].T, KD),
                         _ikp(shared_up_w[hf].T, KD),
                         _ikp(shared_down_w[:, hf].T, KF)))

    x2dT = np.ascontiguousarray(x2d.T)  # [D, T]
    in_maps = []
    idx_list, wt_list, n_list, overflow = [], [], [], []
    for e in range(E):
        rows, slots = np.nonzero(tidx == e)
        n = len(rows)
        if n > R:
            overflow.append((e, rows[R:], slots[R:]))
            rows, slots = rows[:R], slots[:R]
            n = R
        idx_list.append(rows)
        wt_list.append(tw[rows, slots])
        n_list.append(n)
        xe = np.zeros((D, R), np.float32)
        xe[:, :n] = x2dT[:, rows]
        w1e, w3e, w2e = wsets_e[e]
        im = {
            "s0pack": np.concatenate([w1e, w3e, _ikp(xe[:, :n0], KD)], axis=2),
            "xshd": _ikp(x2dT[:, e * SHT:(e + 1) * SHT], KD),
            "w2_0": w2e,
        }
        if R > n0:
            im["xrd"] = _ikp(xe[:, n0:], KD)
        for s, ws in ((1, wsets_sh[0]), (2, wsets_sh[1])):
            im[f"w1_{s}"], im[f"w3_{s}"], im[f"w2_{s}"] = ws
        in_maps.append(im)

    nc = _get_nc(R)
    res = bass_utils.run_bass_kernel_spmd(
        nc, in_maps, core_ids=list(range(N_CORES))
    )
    _CACHE["last_res"] = res

    y = np.zeros((T, D), np.float32)
    for e in range(E):
        n = n_list[e]
        rows = idx_list[e]
        wts = wt_list[e]
        out = res.results[e]
        for j, (mode, s, c0, nj) in enumerate(chunks):
            if mode == "s0":
                continue
            # y{j} is [P, KD, nj]; row d = md*P + p
            blk = out[f"y{j}"].transpose(1, 0, 2).reshape(D, nj)
            if mode == "e":
                lo, hi = c0, min(c0 + nj, n)
                if hi > lo:
                    cols = blk[:, 0:hi - lo].T  # [tokens, D]
                    y[rows[lo:hi]] += wts[lo:hi, None] * cols
            else:  # s1: shared output for token slice
                sl = slice(e * SHT + c0, e * SHT + c0 + nj)
                y[sl] += blk.T
    for e, rows, slots in overflow:
        y[rows] += tw[rows, slots][:, None] * _ffn_host(x2d[rows], w1[e], w2[e], w3[e])

    return y.reshape(B, S, D)


# revision 18
# speedup vs baseline: 1.1696x; 1.0171x over previous
"""Expert-parallel MoE (Kimi/DeepSeek-V3 sparse block) on 8 trn2 NeuronCores.

Strategy (v7):
  - Host computes the sigmoid gate + group-limited top-2 routing in float64
    and gathers each expert's tokens into a transposed, k-interleaved batch.
  - Capacity R = align8(max expert load) is chosen from the *actual*
    routing, so per-core work is R + 2*512 shared token-instances instead of
    a fixed 1536 + 1024 — the program is compiled per capacity and cached.
  - Core e runs expert e's FFN over its R-token batch plus the full shared
    expert (SH=1024, one pass) over token slice [512e : 512(e+1)].
  - All matmul operands are bf16 (fp32 PSUM accumulate): halves HBM->SBUF
    traffic; rel-err ~4e-3 vs the 2e-2 gate.
  - Every DMA moves >=2KB contiguous per partition (the DMA-efficiency
    knee): the first chunk's weights+x are host-packed into one interleaved
    tensor streamed at k-pair granularity; later sets move whole-matrix.
  - Chunk order E0, SH, E1, ..., Er: shared weights arrive during E0; the
    small expert remainder lands last so the post-matmul tail is short.
  - Up-phase mf-outer (k-outer for the streaming first chunk), down-phase
    kf-outer in two md-halves with per-half output DMAs (outputs flush
    while the second half computes); PSUM->SBUF evacuation alternates
    vector/scalar; the final DMAs use both gpsimd and scalar queues.
  - N=512 warm-up matmuls on a zeroed tile run during the initial DMA wait
    so the PE's HAM clock gate is already 8/8 when real work starts.
"""

from contextlib import ExitStack

import numpy as np
import ml_dtypes

import concourse.bacc as bacc
import concourse.tile as tile
import concourse.mybir as mybir
from concourse import bass_utils

# --- model dims (hardcoded per problem spec) ---
B, S, D = 2, 2048, 1024
T = B * S                 # 4096 tokens
E, F = 8, 512             # routed experts / expert intermediate
SH = 1024                 # shared intermediate
TOP_K, N_GROUP, TOPK_GROUP = 2, 4, 2
SCALE = 2.5

N_CORES = 8
P = 128                   # SBUF partitions
KD = D // P               # 8 contraction tiles over D
KF = F // P               # 4 F-tiles per expert
KS = SH // P              # 8 F-tiles for the shared expert
NT = 512                  # max matmul moving free dim (one PSUM bank fp32)
SHT = T // N_CORES        # 512 shared-expert tokens per core
R_MAX = 4096              # capacity ceiling (SBUF); overflow -> host
N_WARM = 10               # N=512 HAM warm-up matmuls (~3.5us busy)

F32 = mybir.dt.float32
BF16 = mybir.dt.bfloat16
BF16_NP = ml_dtypes.bfloat16

_CACHE: dict = {}


def _chunk_plan(R):
    """Chunks: (mode, col_off, ncols); mode 'e' (expert) or 'sh' (shared).

    Order: first expert chunk (k-pair DMA streaming), then the one-pass
    shared chunk, then remaining expert chunks — the small expert remainder
    lands last so the post-matmul tail (copies + output DMA) is short."""
    echunks = []
    c = 0
    while c < R:
        n = min(NT, R - c)
        echunks.append(("e", c, n))
        c += n
    schunks = []
    c = 0
    while c < SHT:
        n = min(NT, SHT - c)
        schunks.append(("sh", c, n))
        c += n
    return tuple(echunks[:1] + schunks + echunks[1:])


def _emit(nc, R):
    chunks = _chunk_plan(R)
    n0 = chunks[0][2]  # first chunk size
    W0 = 2 * F + n0    # packed set0 row: w1 | w3 | x0 per k

    s0d = nc.dram_tensor("s0pack", [P, KD, W0], BF16, kind="ExternalInput").ap()
    xshd = nc.dram_tensor("xshd", [P, KD, SHT], BF16, kind="ExternalInput").ap()
    xrd = (nc.dram_tensor("xrd", [P, KD, R - n0], BF16, kind="ExternalInput").ap()
           if R > n0 else None)
    w1shd = nc.dram_tensor("w1_sh", [P, KD, SH], BF16, kind="ExternalInput").ap()
    w3shd = nc.dram_tensor("w3_sh", [P, KD, SH], BF16, kind="ExternalInput").ap()
    w2d = [nc.dram_tensor("w2_0", [P, KF, D], BF16, kind="ExternalInput").ap(),
           nc.dram_tensor("w2_sh", [P, KS, D], BF16, kind="ExternalInput").ap()]
    youts = [nc.dram_tensor(f"y{j}", [P, KD, n], F32, kind="ExternalOutput").ap()
             for j, (mode, c0, n) in enumerate(chunks)]

    silu = mybir.ActivationFunctionType.Silu

    with tile.TileContext(nc) as tc, ExitStack() as ctx:
        wpool = ctx.enter_context(tc.tile_pool(name="wpool", bufs=1))
        xpool = ctx.enter_context(tc.tile_pool(name="xpool", bufs=1))
        hpool = ctx.enter_context(tc.tile_pool(name="hpool", bufs=2))
        opool = ctx.enter_context(tc.tile_pool(name="opool", bufs=2))
        pspool = ctx.enter_context(tc.tile_pool(name="pspool", bufs=1, space="PSUM"))

        ps_tags = [f"ps{i}" for i in range(8)]

        # ---- SBUF weight tiles: set0 packed at k-pair granularity
        s0p = [wpool.tile([P, 2, W0], BF16, name=f"s0p{j}") for j in range(KD // 2)]
        w2t = [wpool.tile([P, KF, D], BF16, name="w2t0"),
               wpool.tile([P, KS, D], BF16, name="w2tsh")]
        w1sh = wpool.tile([P, KD, SH], BF16, name="w1sh")
        w3sh = wpool.tile([P, KD, SH], BF16, name="w3sh")

        xsh = xpool.tile([P, KD, SHT], BF16, name="xsh")
        xrest = (xpool.tile([P, KD, R - n0], BF16, name="xrest")
                 if R > n0 else None)

        # ---- HAM warm-up: N=512 matmuls on a zeroed tile during DMA wait
        wu = xpool.tile([P, NT], BF16, name="wu")
        nc.vector.memset(wu[:], 0)
        wups = pspool.tile([P, NT], F32, name="wups", tag=ps_tags[7])
        for _ in range(N_WARM):
            nc.tensor.matmul(wups[0:64, :], wu[:, 0:64], wu[:],
                             start=True, stop=True)

        # ---- input DMA stream (sync queue), in consumption order
        for j in range(KD // 2):
            nc.sync.dma_start(s0p[j][:], s0d[:, 2 * j:2 * j + 2, :])
        nc.sync.dma_start(w2t[0][:], w2d[0][:])
        nc.sync.dma_start(w1sh[:], w1shd[:])
        nc.sync.dma_start(xsh[:], xshd[:])
        nc.sync.dma_start(w3sh[:], w3shd[:])
        nc.sync.dma_start(w2t[1][:], w2d[1][:])
        if xrest is not None:
            nc.sync.dma_start(xrest[:], xrd[:])

        def wsl(which, mode, k, blk):
            if mode == "e":
                off = 0 if which == 1 else F
                return s0p[k // 2][:, k % 2, off + blk * P:off + (blk + 1) * P]
            t = w1sh if which == 1 else w3sh
            return t[:, k, blk * P:(blk + 1) * P]

        def xsrc(ci, mode, k, c0, n):
            if mode == "sh":
                return xsh[:, k, c0:c0 + n]
            if ci == 0:
                return s0p[k // 2][:, k % 2, 2 * F:2 * F + n]
            return xrest[:, k, c0 - n0:c0 - n0 + n]

        last_ci = len(chunks) - 1

        def ffn(ci, mode, c0, n):
            yj = youts[ci]
            kfs = KF if mode == "e" else KS
            w2 = w2t[0 if mode == "e" else 1]
            h1s, h3s, hts = [None] * kfs, [None] * kfs, [None] * kfs

            def act_mul(mf):
                a = hpool.tile([P, NT], F32, name="asb", tag="silu")
                nc.scalar.activation(a[:, 0:n], h1s[mf][:, 0:n], silu)
                ht = hpool.tile([P, NT], BF16, name="htsb", tag=f"ht{mf}")
                nc.vector.tensor_mul(ht[:, 0:n], a[:, 0:n], h3s[mf][:, 0:n])
                hts[mf] = ht

            if ci == 0:
                # k-outer: consume weight/x k-pairs in DMA arrival order
                for mf in range(kfs):
                    h1s[mf] = pspool.tile([P, NT], F32, name="h1ps", tag=ps_tags[2 * mf])
                    h3s[mf] = pspool.tile([P, NT], F32, name="h3ps", tag=ps_tags[2 * mf + 1])
                for k in range(KD):
                    xs = xsrc(ci, mode, k, c0, n)
                    st, sp = (k == 0), (k == KD - 1)
                    for mf in range(kfs):
                        nc.tensor.matmul(h1s[mf][:, 0:n], wsl(1, mode, k, mf),
                                         xs, start=st, stop=sp)
                        nc.tensor.matmul(h3s[mf][:, 0:n], wsl(3, mode, k, mf),
                                         xs, start=st, stop=sp)
                for mf in range(kfs):
                    act_mul(mf)
            else:
                # mf-outer: act/mul of pair mf overlaps matmuls of pair mf+1
                for mf in range(kfs):
                    h1s[mf] = pspool.tile([P, NT], F32, name="h1ps",
                                          tag=ps_tags[(2 * mf) % 8])
                    for k in range(KD):
                        nc.tensor.matmul(h1s[mf][:, 0:n], wsl(1, mode, k, mf),
                                         xsrc(ci, mode, k, c0, n),
                                         start=(k == 0), stop=(k == KD - 1))
                    h3s[mf] = pspool.tile([P, NT], F32, name="h3ps",
                                          tag=ps_tags[(2 * mf + 1) % 8])
                    for k in range(KD):
                        nc.tensor.matmul(h3s[mf][:, 0:n], wsl(3, mode, k, mf),
                                         xsrc(ci, mode, k, c0, n),
                                         start=(k == 0), stop=(k == KD - 1))
                    act_mul(mf)

            # down-phase in two md-halves (kf-outer inside each) so the
            # first half's outputs flush while the second half computes
            copy_eng = [nc.vector.tensor_copy, nc.scalar.copy]
            ysb = opool.tile([P, KD, NT], F32, name="ysb", tag="ysb")
            H = KD // 2
            for half in range(2):
                mds = range(half * H, (half + 1) * H)
                yps = {md: pspool.tile([P, NT], F32, name="yps", tag=ps_tags[md])
                       for md in mds}
                for kf in range(kfs):
                    st, sp = (kf == 0), (kf == kfs - 1)
                    for md in mds:
                        nc.tensor.matmul(yps[md][:, 0:n],
                                         w2[:, kf, md * P:(md + 1) * P],
                                         hts[kf][:, 0:n], start=st, stop=sp)
                hsl = slice(half * H, (half + 1) * H)
                for md in mds:
                    copy_eng[md % 2](ysb[:, md, 0:n], yps[md][:, 0:n])
                dma_eng = nc.scalar if (ci == last_ci and half == 1) else nc.gpsimd
                dma_eng.dma_start(yj[:, hsl, :], ysb[:, hsl, 0:n])

        for ci, (mode, c0, n) in enumerate(chunks):
            ffn(ci, mode, c0, n)


def _get_nc(R):
    key = ("nc", R)
    if key not in _CACHE:
        nc = bacc.Bacc("TRN2", target_bir_lowering=False, debug=False,
                       num_devices=N_CORES)
        _emit(nc, R)
        nc.compile()
        _CACHE[key] = nc
    return _CACHE[key]


def _gate_numpy(x2d, gate_w, gate_bias):
    """Replicates reference _moe_gate in float64 (routing-stable)."""
    xl = x2d.astype(np.float64)
    logits = xl @ gate_w.astype(np.float64).T
    scores = 1.0 / (1.0 + np.exp(-logits))
    sc = scores + gate_bias.astype(np.float64)[None, :]
    grp = sc.reshape(T, N_GROUP, E // N_GROUP)
    group_scores = np.sort(grp, axis=-1)[:, :, -2:].sum(-1)
    gidx = np.argsort(-group_scores, axis=-1, kind="stable")[:, :TOPK_GROUP]
    gmask = np.zeros((T, N_GROUP), bool)
    gmask[np.arange(T)[:, None], gidx] = True
    smask = np.repeat(gmask, E // N_GROUP, axis=1)
    tmp = np.where(smask, sc, 0.0)
    tidx = np.argsort(-tmp, axis=-1, kind="stable")[:, :TOP_K]
    tw = np.take_along_axis(scores, tidx, axis=1)
    tw = tw / (tw.sum(-1, keepdims=True) + 1e-20)
    return tidx, (tw * SCALE).astype(np.float32)


def _ffn_host(x, w1e, w2e, w3e):
    """Host fallback for capacity-overflow tokens (pathological skew only)."""
    h = x @ w1e.T
    h = (h / (1.0 + np.exp(-h))) * (x @ w3e.T)
    return h @ w2e.T


def _ikp(mat, kt):
    """[kt*P, X] -> [P, kt, X] bf16 (k-interleaved, partition-major)."""
    return np.asarray(mat).reshape(kt, P, -1).transpose(1, 0, 2).astype(BF16_NP)


def kernel(hidden_states, gate_w, gate_bias, w1, w2, w3,
           shared_gate_w, shared_up_w, shared_down_w):
    hidden_states = np.ascontiguousarray(np.asarray(hidden_states, np.float32))
    gate_w = np.asarray(gate_w, np.float32)
    gate_bias = np.asarray(gate_bias, np.float32)
    w1 = np.asarray(w1, np.float32)
    w2 = np.asarray(w2, np.float32)
    w3 = np.asarray(w3, np.float32)
    shared_gate_w = np.asarray(shared_gate_w, np.float32)
    shared_up_w = np.asarray(shared_up_w, np.float32)
    shared_down_w = np.asarray(shared_down_w, np.float32)

    x2d = hidden_states.reshape(T, D)
    tidx, tw = _gate_numpy(x2d, gate_w, gate_bias)

    counts = np.bincount(tidx.ravel(), minlength=E)
    R = int(min(-(-counts.max() // 8) * 8, R_MAX))
    R = max(R, 8)
    chunks = _chunk_plan(R)
    n0 = chunks[0][2]

    w1sh_i = _ikp(shared_gate_w.T, KD)
    w3sh_i = _ikp(shared_up_w.T, KD)
    w2sh_i = _ikp(shared_down_w.T, KS)

    x2dT = np.ascontiguousarray(x2d.T)  # [D, T]
    in_maps = []
    idx_list, wt_list, n_list, overflow = [], [], [], []
    for e in range(E):
        rows, slots = np.nonzero(tidx == e)
        n = len(rows)
        if n > R:
            overflow.append((e, rows[R:], slots[R:]))
            rows, slots = rows[:R], slots[:R]
            n = R
        idx_list.append(rows)
        wt_list.append(tw[rows, slots])
        n_list.append(n)
        xe = np.zeros((D, R), np.float32)
        xe[:, :n] = x2dT[:, rows]
        im = {
            "s0pack": np.concatenate([_ikp(w1[e].T, KD), _ikp(w3[e].T, KD),
                                      _ikp(xe[:, :n0], KD)], axis=2),
            "xshd": _ikp(x2dT[:, e * SHT:(e + 1) * SHT], KD),
            "w2_0": _ikp(w2[e].T, KF),
            "w1_sh": w1sh_i, "w3_sh": w3sh_i, "w2_sh": w2sh_i,
        }
        if R > n0:
            im["xrd"] = _ikp(xe[:, n0:], KD)
        in_maps.append(im)

    nc = _get_nc(R)
    res = bass_utils.run_bass_kernel_spmd(
        nc, in_maps, core_ids=list(range(N_CORES))
    )
    _CACHE["last_res"] = res

    y = np.zeros((T, D), np.float32)
    for e in range(E):
        n = n_list[e]
        rows = idx_list[e]
        wts = wt_list[e]
        out = res.results[e]
        for j, (mode, c0, nj) in enumerate(chunks):
            # y{j} is [P, KD, nj]; row d = md*P + p
            blk = out[f"y{j}"].transpose(1, 0, 2).reshape(D, nj)
            if mode == "e":
                lo, hi = c0, min(c0 + nj, n)
                if hi > lo:
                    y[rows[lo:hi]] += wts[lo:hi, None] * blk[:, 0:hi - lo].T
            else:  # shared output for token slice
                sl = slice(e * SHT + c0, e * SHT + c0 + nj)
                y[sl] += blk.T
    for e, rows, slots in overflow:
        y[rows] += tw[rows, slots][:, None] * _ffn_host(x2d[rows], w1[e], w2[e], w3[e])

    return y.reshape(B, S, D)


# revision 20
# speedup vs baseline: 1.1749x; 1.0046x over previous
"""Expert-parallel MoE (Kimi/DeepSeek-V3 sparse block) on 8 trn2 NeuronCores.

Strategy (v7):
  - Host computes the sigmoid gate + group-limited top-2 routing in float64
    and gathers each expert's tokens into a transposed, k-interleaved batch.
  - Capacity R = align8(max expert load) is chosen from the *actual*
    routing, so per-core work is R + 2*512 shared token-instances instead of
    a fixed 1536 + 1024 — the program is compiled per capacity and cached.
  - Core e runs expert e's FFN over its R-token batch plus the full shared
    expert (SH=1024, one pass) over token slice [512e : 512(e+1)].
  - All matmul operands are bf16 (fp32 PSUM accumulate): halves HBM->SBUF
    traffic; rel-err ~4e-3 vs the 2e-2 gate.
  - Every DMA moves >=2KB contiguous per partition (the DMA-efficiency
    knee): the first chunk's weights+x are host-packed into one interleaved
    tensor streamed at k-pair granularity; later sets move whole-matrix.
  - Chunk order E0, SH, E1, ..., Er: shared weights arrive during E0; the
    small expert remainder lands last so the post-matmul tail is short.
  - Up-phase mf-outer (k-outer for the streaming first chunk), down-phase
    kf-outer in two md-halves with per-half output DMAs (outputs flush
    while the second half computes); PSUM->SBUF evacuation alternates
    vector/scalar; the final DMAs use both gpsimd and scalar queues.
  - N=512 warm-up matmuls on a zeroed tile run during the initial DMA wait
    so the PE's HAM clock gate is already 8/8 when real work starts.
"""

from contextlib import ExitStack

import numpy as np
import ml_dtypes

import concourse.bacc as bacc
import concourse.tile as tile
import concourse.mybir as mybir
from concourse import bass_utils

# --- model dims (hardcoded per problem spec) ---
B, S, D = 2, 2048, 1024
T = B * S                 # 4096 tokens
E, F = 8, 512             # routed experts / expert intermediate
SH = 1024                 # shared intermediate
TOP_K, N_GROUP, TOPK_GROUP = 2, 4, 2
SCALE = 2.5

N_CORES = 8
P = 128                   # SBUF partitions
KD = D // P               # 8 contraction tiles over D
KF = F // P               # 4 F-tiles per expert
KS = SH // P              # 8 F-tiles for the shared expert
NT = 512                  # max matmul moving free dim (one PSUM bank fp32)
SHT = T // N_CORES        # 512 shared-expert tokens per core
R_MAX = 4096              # capacity ceiling (SBUF); overflow -> host
N_WARM = 10               # N=512 HAM warm-up matmuls (~3.5us busy)

F32 = mybir.dt.float32
BF16 = mybir.dt.bfloat16
BF16_NP = ml_dtypes.bfloat16

_CACHE: dict = {}


def _chunk_plan(R):
    """Chunks: (mode, col_off, ncols); mode 'e' (expert) or 'sh' (shared).

    Order: first expert chunk (k-pair DMA streaming), then the one-pass
    shared chunk, then remaining expert chunks — the small expert remainder
    lands last so the post-matmul tail (copies + output DMA) is short."""
    echunks = []
    c = 0
    while c < R:
        n = min(NT, R - c)
        echunks.append(("e", c, n))
        c += n
    schunks = []
    c = 0
    while c < SHT:
        n = min(NT, SHT - c)
        schunks.append(("sh", c, n))
        c += n
    return tuple(echunks[:1] + schunks + echunks[1:])


def _emit(nc, R):
    chunks = _chunk_plan(R)
    n0 = chunks[0][2]  # first chunk size
    W0 = 2 * F + n0    # packed set0 row: w1 | w3 | x0 per k

    s0d = nc.dram_tensor("s0pack", [P, KD, W0], BF16, kind="ExternalInput").ap()
    xshd = nc.dram_tensor("xshd", [P, KD, SHT], BF16, kind="ExternalInput").ap()
    xrd = (nc.dram_tensor("xrd", [P, KD, R - n0], BF16, kind="ExternalInput").ap()
           if R > n0 else None)
    w1shd = nc.dram_tensor("w1_sh", [P, KD, SH], BF16, kind="ExternalInput").ap()
    w3shd = nc.dram_tensor("w3_sh", [P, KD, SH], BF16, kind="ExternalInput").ap()
    w2d = [nc.dram_tensor("w2_0", [P, KF, D], BF16, kind="ExternalInput").ap(),
           nc.dram_tensor("w2_sh", [P, KS, D], BF16, kind="ExternalInput").ap()]
    youts = [nc.dram_tensor(f"y{j}", [P, KD, n], F32, kind="ExternalOutput").ap()
             for j, (mode, c0, n) in enumerate(chunks)]

    silu = mybir.ActivationFunctionType.Silu

    with tile.TileContext(nc) as tc, ExitStack() as ctx:
        wpool = ctx.enter_context(tc.tile_pool(name="wpool", bufs=1))
        xpool = ctx.enter_context(tc.tile_pool(name="xpool", bufs=1))
        hpool = ctx.enter_context(tc.tile_pool(name="hpool", bufs=2))
        opool = ctx.enter_context(tc.tile_pool(name="opool", bufs=2))
        pspool = ctx.enter_context(tc.tile_pool(name="pspool", bufs=1, space="PSUM"))

        ps_tags = [f"ps{i}" for i in range(8)]

        # ---- SBUF weight tiles: set0 packed at k-pair granularity
        s0p = [wpool.tile([P, 2, W0], BF16, name=f"s0p{j}") for j in range(KD // 2)]
        w2t = [wpool.tile([P, KF, D], BF16, name="w2t0"),
               wpool.tile([P, KS, D], BF16, name="w2tsh")]
        w1sh = wpool.tile([P, KD, SH], BF16, name="w1sh")
        w3sh = wpool.tile([P, KD, SH], BF16, name="w3sh")

        xsh = xpool.tile([P, KD, SHT], BF16, name="xsh")
        xrest = (xpool.tile([P, KD, R - n0], BF16, name="xrest")
                 if R > n0 else None)

        # ---- HAM warm-up: N=512 matmuls on a zeroed tile during DMA wait
        wu = xpool.tile([P, NT], BF16, name="wu")
        nc.vector.memset(wu[:], 0)
        wups = pspool.tile([P, NT], F32, name="wups", tag=ps_tags[7])
        for _ in range(N_WARM):
            nc.tensor.matmul(wups[0:64, :], wu[:, 0:64], wu[:],
                             start=True, stop=True)

        # ---- input DMA stream (sync queue), in consumption order
        # first k-pair in two halves so the very first matmul starts sooner
        nc.sync.dma_start(s0p[0][:, 0, :], s0d[:, 0, :])
        nc.sync.dma_start(s0p[0][:, 1, :], s0d[:, 1, :])
        for j in range(1, KD // 2):
            nc.sync.dma_start(s0p[j][:], s0d[:, 2 * j:2 * j + 2, :])
        nc.sync.dma_start(w2t[0][:], w2d[0][:])
        nc.sync.dma_start(w1sh[:], w1shd[:])
        nc.sync.dma_start(xsh[:], xshd[:])
        nc.sync.dma_start(w3sh[:], w3shd[:])
        nc.sync.dma_start(w2t[1][:], w2d[1][:])
        if xrest is not None:
            nc.sync.dma_start(xrest[:], xrd[:])

        def wsl(which, mode, k, blk):
            if mode == "e":
                off = 0 if which == 1 else F
                return s0p[k // 2][:, k % 2, off + blk * P:off + (blk + 1) * P]
            t = w1sh if which == 1 else w3sh
            return t[:, k, blk * P:(blk + 1) * P]

        def xsrc(ci, mode, k, c0, n):
            if mode == "sh":
                return xsh[:, k, c0:c0 + n]
            if ci == 0:
                return s0p[k // 2][:, k % 2, 2 * F:2 * F + n]
            return xrest[:, k, c0 - n0:c0 - n0 + n]

        last_ci = len(chunks) - 1

        def ffn(ci, mode, c0, n):
            yj = youts[ci]
            kfs = KF if mode == "e" else KS
            w2 = w2t[0 if mode == "e" else 1]
            h1s, h3s, hts = [None] * kfs, [None] * kfs, [None] * kfs

            def act_mul(mf):
                a = hpool.tile([P, NT], F32, name="asb", tag="silu")
                nc.scalar.activation(a[:, 0:n], h1s[mf][:, 0:n], silu)
                ht = hpool.tile([P, NT], BF16, name="htsb", tag=f"ht{mf}")
                nc.vector.tensor_mul(ht[:, 0:n], a[:, 0:n], h3s[mf][:, 0:n])
                hts[mf] = ht

            if ci == 0:
                # k-outer: consume weight/x k-pairs in DMA arrival order
                for mf in range(kfs):
                    h1s[mf] = pspool.tile([P, NT], F32, name="h1ps", tag=ps_tags[2 * mf])
                    h3s[mf] = pspool.tile([P, NT], F32, name="h3ps", tag=ps_tags[2 * mf + 1])
                for k in range(KD):
                    xs = xsrc(ci, mode, k, c0, n)
                    st, sp = (k == 0), (k == KD - 1)
                    for mf in range(kfs):
                        nc.tensor.matmul(h1s[mf][:, 0:n], wsl(1, mode, k, mf),
                                         xs, start=st, stop=sp)
                        nc.tensor.matmul(h3s[mf][:, 0:n], wsl(3, mode, k, mf),
                                         xs, start=st, stop=sp)
                for mf in range(kfs):
                    act_mul(mf)
            else:
                # mf-outer: act/mul of pair mf overlaps matmuls of pair mf+1
                for mf in range(kfs):
                    h1s[mf] = pspool.tile([P, NT], F32, name="h1ps",
                                          tag=ps_tags[(2 * mf) % 8])
                    for k in range(KD):
                        nc.tensor.matmul(h1s[mf][:, 0:n], wsl(1, mode, k, mf),
                                         xsrc(ci, mode, k, c0, n),
                                         start=(k == 0), stop=(k == KD - 1))
                    h3s[mf] = pspool.tile([P, NT], F32, name="h3ps",
                                          tag=ps_tags[(2 * mf + 1) % 8])
                    for k in range(KD):
                        nc.tensor.matmul(h3s[mf][:, 0:n], wsl(3, mode, k, mf),
                                         xsrc(ci, mode, k, c0, n),
                                         start=(k == 0), stop=(k == KD - 1))
                    act_mul(mf)

            # down-phase in two md-halves (kf-outer inside each) so the
            # first half's outputs flush while the second half computes
            copy_eng = [nc.vector.tensor_copy, nc.scalar.copy]
            ysb = opool.tile([P, KD, NT], F32, name="ysb", tag="ysb")
            H = KD // 2
            for half in range(2):
                mds = range(half * H, (half + 1) * H)
                yps = {md: pspool.tile([P, NT], F32, name="yps", tag=ps_tags[md])
                       for md in mds}
                for kf in range(kfs):
                    st, sp = (kf == 0), (kf == kfs - 1)
                    for md in mds:
                        nc.tensor.matmul(yps[md][:, 0:n],
                                         w2[:, kf, md * P:(md + 1) * P],
                                         hts[kf][:, 0:n], start=st, stop=sp)
                hsl = slice(half * H, (half + 1) * H)
                for md in mds:
                    copy_eng[md % 2](ysb[:, md, 0:n], yps[md][:, 0:n])
                dma_eng = nc.gpsimd if half == 0 else nc.scalar
                dma_eng.dma_start(yj[:, hsl, :], ysb[:, hsl, 0:n])

        for ci, (mode, c0, n) in enumerate(chunks):
            ffn(ci, mode, c0, n)


def _get_nc(R):
    key = ("nc", R)
    if key not in _CACHE:
        nc = bacc.Bacc("TRN2", target_bir_lowering=False, debug=False,
                       num_devices=N_CORES)
        _emit(nc, R)
        nc.compile()
        _CACHE[key] = nc
    return _CACHE[key]


def _gate_numpy(x2d, gate_w, gate_bias):
    """Replicates reference _moe_gate in float64 (routing-stable)."""
    xl = x2d.astype(np.float64)
    logits = xl @ gate_w.astype(np.float64).T
    scores = 1.0 / (1.0 + np.exp(-logits))
    sc = scores + gate_bias.astype(np.float64)[None, :]
    grp = sc.reshape(T, N_GROUP, E // N_GROUP)
    group_scores = np.sort(grp, axis=-1)[:, :, -2:].sum(-1)
    gidx = np.argsort(-group_scores, axis=-1, kind="stable")[:, :TOPK_GROUP]
    gmask = np.zeros((T, N_GROUP), bool)
    gmask[np.arange(T)[:, None], gidx] = True
    smask = np.repeat(gmask, E // N_GROUP, axis=1)
    tmp = np.where(smask, sc, 0.0)
    tidx = np.argsort(-tmp, axis=-1, kind="stable")[:, :TOP_K]
    tw = np.take_along_axis(scores, tidx, axis=1)
    tw = tw / (tw.sum(-1, keepdims=True) + 1e-20)
    return tidx, (tw * SCALE).astype(np.float32)


def _ffn_host(x, w1e, w2e, w3e):
    """Host fallback for capacity-overflow tokens (pathological skew only)."""
    h = x @ w1e.T
    h = (h / (1.0 + np.exp(-h))) * (x @ w3e.T)
    return h @ w2e.T


def _ikp(mat, kt):
    """[kt*P, X] -> [P, kt, X] bf16 (k-interleaved, partition-major)."""
    return np.asarray(mat).reshape(kt, P, -1).transpose(1, 0, 2).astype(BF16_NP)


def kernel(hidden_states, gate_w, gate_bias, w1, w2, w3,
           shared_gate_w, shared_up_w, shared_down_w):
    hidden_states = np.ascontiguousarray(np.asarray(hidden_states, np.float32))
    gate_w = np.asarray(gate_w, np.float32)
    gate_bias = np.asarray(gate_bias, np.float32)
    w1 = np.asarray(w1, np.float32)
    w2 = np.asarray(w2, np.float32)
    w3 = np.asarray(w3, np.float32)
    shared_gate_w = np.asarray(shared_gate_w, np.float32)
    shared_up_w = np.asarray(shared_up_w, np.float32)
    shared_down_w = np.asarray(shared_down_w, np.float32)

    x2d = hidden_states.reshape(T, D)
    tidx, tw = _gate_numpy(x2d, gate_w, gate_bias)

    counts = np.bincount(tidx.ravel(), minlength=E)
    R = int(min(-(-counts.max() // 8) * 8, R_MAX))
    R = max(R, 8)
    chunks = _chunk_plan(R)
    n0 = chunks[0][2]

    w1sh_i = _ikp(shared_gate_w.T, KD)
    w3sh_i = _ikp(shared_up_w.T, KD)
    w2sh_i = _ikp(shared_down_w.T, KS)

    x2dT = np.ascontiguousarray(x2d.T)  # [D, T]
    in_maps = []
    idx_list, wt_list, n_list, overflow = [], [], [], []
    for e in range(E):
        rows, slots = np.nonzero(tidx == e)
        n = len(rows)
        if n > R:
            overflow.append((e, rows[R:], slots[R:]))
            rows, slots = rows[:R], slots[:R]
            n = R
        idx_list.append(rows)
        wt_list.append(tw[rows, slots])
        n_list.append(n)
        xe = np.zeros((D, R), np.float32)
        xe[:, :n] = x2dT[:, rows]
        im = {
            "s0pack": np.concatenate([_ikp(w1[e].T, KD), _ikp(w3[e].T, KD),
                                      _ikp(xe[:, :n0], KD)], axis=2),
            "xshd": _ikp(x2dT[:, e * SHT:(e + 1) * SHT], KD),
            "w2_0": _ikp(w2[e].T, KF),
            "w1_sh": w1sh_i, "w3_sh": w3sh_i, "w2_sh": w2sh_i,
        }
        if R > n0:
            im["xrd"] = _ikp(xe[:, n0:], KD)
        in_maps.append(im)

    nc = _get_nc(R)
    res = bass_utils.run_bass_kernel_spmd(
        nc, in_maps, core_ids=list(range(N_CORES))
    )
    _CACHE["last_res"] = res

    y = np.zeros((T, D), np.float32)
    for e in range(E):
        n = n_list[e]
        rows = idx_list[e]
        wts = wt_list[e]
        out = res.results[e]
        for j, (mode, c0, nj) in enumerate(chunks):
            # y{j} is [P, KD, nj]; row d = md*P + p
            blk = out[f"y{j}"].transpose(1, 0, 2).reshape(D, nj)
            if mode == "e":
                lo, hi = c0, min(c0 + nj, n)
                if hi > lo:
                    y[rows[lo:hi]] += wts[lo:hi, None] * blk[:, 0:hi - lo].T
            else:  # shared output for token slice
                sl = slice(e * SHT + c0, e * SHT + c0 + nj)
                y[sl] += blk.T
    for e, rows, slots in overflow:
        y[rows] += tw[rows, slots][:, None] * _ffn_host(x2d[rows], w1[e], w2[e], w3[e])

    return y.reshape(B, S, D)


# revision 21
# speedup vs baseline: 1.1906x; 1.0133x over previous
"""Expert-parallel MoE (Kimi/DeepSeek-V3 sparse block) on 8 trn2 NeuronCores.

Strategy (v7):
  - Host computes the sigmoid gate + group-limited top-2 routing in float64
    and gathers each expert's tokens into a transposed, k-interleaved batch.
  - Capacity R = align8(max expert load) is chosen from the *actual*
    routing, so per-core work is R + 2*512 shared token-instances instead of
    a fixed 1536 + 1024 — the program is compiled per capacity and cached.
  - Core e runs expert e's FFN over its R-token batch plus the full shared
    expert (SH=1024, one pass) over token slice [512e : 512(e+1)].
  - All matmul operands are bf16 (fp32 PSUM accumulate): halves HBM->SBUF
    traffic; rel-err ~4e-3 vs the 2e-2 gate.
  - Every DMA moves >=2KB contiguous per partition (the DMA-efficiency
    knee): the first chunk's weights+x are host-packed into one interleaved
    tensor streamed at k-pair granularity; later sets move whole-matrix.
  - Chunk order E0, SH, E1, ..., Er: shared weights arrive during E0; the
    small expert remainder lands last so the post-matmul tail is short.
  - Up-phase mf-outer (k-outer for the streaming first chunk), down-phase
    kf-outer in two md-halves with per-half output DMAs (outputs flush
    while the second half computes); PSUM->SBUF evacuation alternates
    vector/scalar; the final DMAs use both gpsimd and scalar queues.
  - N=512 warm-up matmuls on a zeroed tile run during the initial DMA wait
    so the PE's HAM clock gate is already 8/8 when real work starts.
"""

from contextlib import ExitStack

import numpy as np
import ml_dtypes

import concourse.bacc as bacc
import concourse.tile as tile
import concourse.mybir as mybir
from concourse import bass_utils

# --- model dims (hardcoded per problem spec) ---
B, S, D = 2, 2048, 1024
T = B * S                 # 4096 tokens
E, F = 8, 512             # routed experts / expert intermediate
SH = 1024                 # shared intermediate
TOP_K, N_GROUP, TOPK_GROUP = 2, 4, 2
SCALE = 2.5

N_CORES = 8
P = 128                   # SBUF partitions
KD = D // P               # 8 contraction tiles over D
KF = F // P               # 4 F-tiles per expert
KS = SH // P              # 8 F-tiles for the shared expert
NT = 512                  # max matmul moving free dim (one PSUM bank fp32)
SHT = T // N_CORES        # 512 shared-expert tokens per core
R_MAX = 4096              # capacity ceiling (SBUF); overflow -> host
N_WARM = 8                # N=512 HAM warm-up matmuls (~3.4us busy)

F32 = mybir.dt.float32
BF16 = mybir.dt.bfloat16
BF16_NP = ml_dtypes.bfloat16

_CACHE: dict = {}


def _chunk_plan(R):
    """Chunks: (mode, col_off, ncols); mode 'e' (expert) or 'sh' (shared).

    Order: first expert chunk (k-pair DMA streaming), then the one-pass
    shared chunk, then remaining expert chunks — the small expert remainder
    lands last so the post-matmul tail (copies + output DMA) is short."""
    echunks = []
    c = 0
    while c < R:
        n = min(NT, R - c)
        echunks.append(("e", c, n))
        c += n
    schunks = []
    c = 0
    while c < SHT:
        n = min(NT, SHT - c)
        schunks.append(("sh", c, n))
        c += n
    return tuple(echunks[:1] + schunks + echunks[1:])


def _emit(nc, R):
    chunks = _chunk_plan(R)
    n0 = chunks[0][2]  # first chunk size
    W0 = 2 * F + n0    # packed set0 row: w1 | w3 | x0 per k

    s0d = nc.dram_tensor("s0pack", [P, KD, W0], BF16, kind="ExternalInput").ap()
    xshd = nc.dram_tensor("xshd", [P, KD, SHT], BF16, kind="ExternalInput").ap()
    xrd = (nc.dram_tensor("xrd", [P, KD, R - n0], BF16, kind="ExternalInput").ap()
           if R > n0 else None)
    w1shd = nc.dram_tensor("w1_sh", [P, KD, SH], BF16, kind="ExternalInput").ap()
    w3shd = nc.dram_tensor("w3_sh", [P, KD, SH], BF16, kind="ExternalInput").ap()
    w2d = [nc.dram_tensor("w2_0", [P, KF, D], BF16, kind="ExternalInput").ap(),
           nc.dram_tensor("w2_sh", [P, KS, D], BF16, kind="ExternalInput").ap()]
    youts = [nc.dram_tensor(f"y{j}", [P, KD, n], BF16, kind="ExternalOutput").ap()
             for j, (mode, c0, n) in enumerate(chunks)]

    silu = mybir.ActivationFunctionType.Silu

    with tile.TileContext(nc) as tc, ExitStack() as ctx:
        wpool = ctx.enter_context(tc.tile_pool(name="wpool", bufs=1))
        xpool = ctx.enter_context(tc.tile_pool(name="xpool", bufs=1))
        hpool = ctx.enter_context(tc.tile_pool(name="hpool", bufs=2))
        opool = ctx.enter_context(tc.tile_pool(name="opool", bufs=2))
        pspool = ctx.enter_context(tc.tile_pool(name="pspool", bufs=1, space="PSUM"))

        ps_tags = [f"ps{i}" for i in range(8)]

        # ---- SBUF weight tiles: set0 packed at k-pair granularity
        s0p = [wpool.tile([P, 2, W0], BF16, name=f"s0p{j}") for j in range(KD // 2)]
        w2t = [wpool.tile([P, KF, D], BF16, name="w2t0"),
               wpool.tile([P, KS, D], BF16, name="w2tsh")]
        w1sh = wpool.tile([P, KD, SH], BF16, name="w1sh")
        w3sh = wpool.tile([P, KD, SH], BF16, name="w3sh")

        xsh = xpool.tile([P, KD, SHT], BF16, name="xsh")
        xrest = (xpool.tile([P, KD, R - n0], BF16, name="xrest")
                 if R > n0 else None)

        # ---- HAM warm-up: N=512 matmuls on a zeroed tile during DMA wait
        wu = xpool.tile([P, NT], BF16, name="wu")
        nc.vector.memset(wu[:], 0)
        wups = pspool.tile([P, NT], F32, name="wups", tag=ps_tags[7])
        for _ in range(N_WARM):
            nc.tensor.matmul(wups[0:64, :], wu[:, 0:64], wu[:],
                             start=True, stop=True)

        # ---- input DMA stream (sync queue), in consumption order
        # first k-pair in two halves so the very first matmul starts sooner
        nc.sync.dma_start(s0p[0][:, 0, :], s0d[:, 0, :])
        nc.sync.dma_start(s0p[0][:, 1, :], s0d[:, 1, :])
        for j in range(1, KD // 2):
            nc.sync.dma_start(s0p[j][:], s0d[:, 2 * j:2 * j + 2, :])
        nc.sync.dma_start(w2t[0][:], w2d[0][:])
        nc.sync.dma_start(w1sh[:], w1shd[:])
        nc.sync.dma_start(xsh[:], xshd[:])
        nc.sync.dma_start(w3sh[:], w3shd[:])
        nc.sync.dma_start(w2t[1][:], w2d[1][:])
        if xrest is not None:
            nc.sync.dma_start(xrest[:], xrd[:])

        def wsl(which, mode, k, blk):
            if mode == "e":
                off = 0 if which == 1 else F
                return s0p[k // 2][:, k % 2, off + blk * P:off + (blk + 1) * P]
            t = w1sh if which == 1 else w3sh
            return t[:, k, blk * P:(blk + 1) * P]

        def xsrc(ci, mode, k, c0, n):
            if mode == "sh":
                return xsh[:, k, c0:c0 + n]
            if ci == 0:
                return s0p[k // 2][:, k % 2, 2 * F:2 * F + n]
            return xrest[:, k, c0 - n0:c0 - n0 + n]

        last_ci = len(chunks) - 1

        def ffn(ci, mode, c0, n):
            yj = youts[ci]
            kfs = KF if mode == "e" else KS
            w2 = w2t[0 if mode == "e" else 1]
            h1s, h3s, hts = [None] * kfs, [None] * kfs, [None] * kfs

            def act_mul(mf):
                a = hpool.tile([P, NT], F32, name="asb", tag="silu")
                nc.scalar.activation(a[:, 0:n], h1s[mf][:, 0:n], silu)
                ht = hpool.tile([P, NT], BF16, name="htsb", tag=f"ht{mf}")
                nc.vector.tensor_mul(ht[:, 0:n], a[:, 0:n], h3s[mf][:, 0:n])
                hts[mf] = ht

            if ci == 0:
                # k-outer: consume weight/x k-pairs in DMA arrival order
                for mf in range(kfs):
                    h1s[mf] = pspool.tile([P, NT], F32, name="h1ps", tag=ps_tags[2 * mf])
                    h3s[mf] = pspool.tile([P, NT], F32, name="h3ps", tag=ps_tags[2 * mf + 1])
                for k in range(KD):
                    xs = xsrc(ci, mode, k, c0, n)
                    st, sp = (k == 0), (k == KD - 1)
                    for mf in range(kfs):
                        nc.tensor.matmul(h1s[mf][:, 0:n], wsl(1, mode, k, mf),
                                         xs, start=st, stop=sp)
                        nc.tensor.matmul(h3s[mf][:, 0:n], wsl(3, mode, k, mf),
                                         xs, start=st, stop=sp)
                for mf in range(kfs):
                    act_mul(mf)
            else:
                # mf-outer: act/mul of pair mf overlaps matmuls of pair mf+1
                for mf in range(kfs):
                    h1s[mf] = pspool.tile([P, NT], F32, name="h1ps",
                                          tag=ps_tags[(2 * mf) % 8])
                    for k in range(KD):
                        nc.tensor.matmul(h1s[mf][:, 0:n], wsl(1, mode, k, mf),
                                         xsrc(ci, mode, k, c0, n),
                                         start=(k == 0), stop=(k == KD - 1))
                    h3s[mf] = pspool.tile([P, NT], F32, name="h3ps",
                                          tag=ps_tags[(2 * mf + 1) % 8])
                    for k in range(KD):
                        nc.tensor.matmul(h3s[mf][:, 0:n], wsl(3, mode, k, mf),
                                         xsrc(ci, mode, k, c0, n),
                                         start=(k == 0), stop=(k == KD - 1))
                    act_mul(mf)

            # down-phase in two md-halves (kf-outer inside each) so the
            # first half's outputs flush while the second half computes
            copy_eng = [nc.vector.tensor_copy, nc.scalar.copy]
            ysb = opool.tile([P, KD, NT], BF16, name="ysb", tag="ysb")
            H = KD // 2
            for half in range(2):
                mds = range(half * H, (half + 1) * H)
                yps = {md: pspool.tile([P, NT], F32, name="yps", tag=ps_tags[md])
                       for md in mds}
                for kf in range(kfs):
                    st, sp = (kf == 0), (kf == kfs - 1)
                    for md in mds:
                        nc.tensor.matmul(yps[md][:, 0:n],
                                         w2[:, kf, md * P:(md + 1) * P],
                                         hts[kf][:, 0:n], start=st, stop=sp)
                hsl = slice(half * H, (half + 1) * H)
                for md in mds:
                    copy_eng[md % 2](ysb[:, md, 0:n], yps[md][:, 0:n])
                dma_eng = nc.gpsimd if half == 0 else nc.scalar
                dma_eng.dma_start(yj[:, hsl, :], ysb[:, hsl, 0:n])

        for ci, (mode, c0, n) in enumerate(chunks):
            ffn(ci, mode, c0, n)


def _get_nc(R):
    key = ("nc", R)
    if key not in _CACHE:
        nc = bacc.Bacc("TRN2", target_bir_lowering=False, debug=False,
                       num_devices=N_CORES)
        _emit(nc, R)
        nc.compile()
        _CACHE[key] = nc
    return _CACHE[key]


def _gate_numpy(x2d, gate_w, gate_bias):
    """Replicates reference _moe_gate in float64 (routing-stable)."""
    xl = x2d.astype(np.float64)
    logits = xl @ gate_w.astype(np.float64).T
    scores = 1.0 / (1.0 + np.exp(-logits))
    sc = scores + gate_bias.astype(np.float64)[None, :]
    grp = sc.reshape(T, N_GROUP, E // N_GROUP)
    group_scores = np.sort(grp, axis=-1)[:, :, -2:].sum(-1)
    gidx = np.argsort(-group_scores, axis=-1, kind="stable")[:, :TOPK_GROUP]
    gmask = np.zeros((T, N_GROUP), bool)
    gmask[np.arange(T)[:, None], gidx] = True
    smask = np.repeat(gmask, E // N_GROUP, axis=1)
    tmp = np.where(smask, sc, 0.0)
    tidx = np.argsort(-tmp, axis=-1, kind="stable")[:, :TOP_K]
    tw = np.take_along_axis(scores, tidx, axis=1)
    tw = tw / (tw.sum(-1, keepdims=True) + 1e-20)
    return tidx, (tw * SCALE).astype(np.float32)


def _ffn_host(x, w1e, w2e, w3e):
    """Host fallback for capacity-overflow tokens (pathological skew only)."""
    h = x @ w1e.T
    h = (h / (1.0 + np.exp(-h))) * (x @ w3e.T)
    return h @ w2e.T


def _ikp(mat, kt):
    """[kt*P, X] -> [P, kt, X] bf16 (k-interleaved, partition-major)."""
    return np.asarray(mat).reshape(kt, P, -1).transpose(1, 0, 2).astype(BF16_NP)


def kernel(hidden_states, gate_w, gate_bias, w1, w2, w3,
           shared_gate_w, shared_up_w, shared_down_w):
    hidden_states = np.ascontiguousarray(np.asarray(hidden_states, np.float32))
    gate_w = np.asarray(gate_w, np.float32)
    gate_bias = np.asarray(gate_bias, np.float32)
    w1 = np.asarray(w1, np.float32)
    w2 = np.asarray(w2, np.float32)
    w3 = np.asarray(w3, np.float32)
    shared_gate_w = np.asarray(shared_gate_w, np.float32)
    shared_up_w = np.asarray(shared_up_w, np.float32)
    shared_down_w = np.asarray(shared_down_w, np.float32)

    x2d = hidden_states.reshape(T, D)
    tidx, tw = _gate_numpy(x2d, gate_w, gate_bias)

    counts = np.bincount(tidx.ravel(), minlength=E)
    R = int(min(-(-counts.max() // 8) * 8, R_MAX))
    R = max(R, 8)
    chunks = _chunk_plan(R)
    n0 = chunks[0][2]

    w1sh_i = _ikp(shared_gate_w.T, KD)
    w3sh_i = _ikp(shared_up_w.T, KD)
    w2sh_i = _ikp(shared_down_w.T, KS)

    x2dT = np.ascontiguousarray(x2d.T)  # [D, T]
    in_maps = []
    idx_list, wt_list, n_list, overflow = [], [], [], []
    for e in range(E):
        rows, slots = np.nonzero(tidx == e)
        n = len(rows)
        if n > R:
            overflow.append((e, rows[R:], slots[R:]))
            rows, slots = rows[:R], slots[:R]
            n = R
        idx_list.append(rows)
        wt_list.append(tw[rows, slots])
        n_list.append(n)
        xe = np.zeros((D, R), np.float32)
        xe[:, :n] = x2dT[:, rows]
        im = {
            "s0pack": np.concatenate([_ikp(w1[e].T, KD), _ikp(w3[e].T, KD),
                                      _ikp(xe[:, :n0], KD)], axis=2),
            "xshd": _ikp(x2dT[:, e * SHT:(e + 1) * SHT], KD),
            "w2_0": _ikp(w2[e].T, KF),
            "w1_sh": w1sh_i, "w3_sh": w3sh_i, "w2_sh": w2sh_i,
        }
        if R > n0:
            im["xrd"] = _ikp(xe[:, n0:], KD)
        in_maps.append(im)

    nc = _get_nc(R)
    res = bass_utils.run_bass_kernel_spmd(
        nc, in_maps, core_ids=list(range(N_CORES))
    )
    _CACHE["last_res"] = res

    y = np.zeros((T, D), np.float32)
    for e in range(E):
        n = n_list[e]
        rows = idx_list[e]
        wts = wt_list[e]
        out = res.results[e]
        for j, (mode, c0, nj) in enumerate(chunks):
            # y{j} is [P, KD, nj]; row d = md*P + p
            blk = np.asarray(out[f"y{j}"], np.float32).transpose(1, 0, 2).reshape(D, nj)
            if mode == "e":
                lo, hi = c0, min(c0 + nj, n)
                if hi > lo:
                    y[rows[lo:hi]] += wts[lo:hi, None] * blk[:, 0:hi - lo].T
            else:  # shared output for token slice
                sl = slice(e * SHT + c0, e * SHT + c0 + nj)
                y[sl] += blk.T
    for e, rows, slots in overflow:
        y[rows] += tw[rows, slots][:, None] * _ffn_host(x2d[rows], w1[e], w2[e], w3[e])

    return y.reshape(B, S, D)
